# revision 1
# baseline (speedup 1.0000x reference)
"""Trainium2 Bass kernel for nn_EncoderBlock (dense transformer block).

Reference computation (fp32, S=2048 B=2 D=1024 H=16 dh=64 F=4096):
    q,k,v = x@Wq+bq, x@Wk+bk, x@Wv+bv          (per-head split, dh=64)
    attn  = softmax(q k^T / sqrt(dh)) v         (full S x S scores)
    o     = attn-merge @ Wo + bo
    x1    = LN(x + o; g1,b1)
    out   = LN(x1 + relu(x1@W1+bb1)@W2+bb2; g2,b2)

Sharding: sequence-parallel over 8 cores. Each core owns 256 seq positions
(x 2 batches = 512 tokens) end-to-end; K/V are computed redundantly on every
core (an on-chip collective costs more than the recompute at this size).

Precision: the Q/K/V projections and the O-projection run in fp8-e4m3
with DoubleRow perf mode (2 k-tiles per instruction, 0.5 cycles/row =>
4x bf16 throughput). QKV noise washes out in the softmax average over
2048 keys; O-proj noise rides on o, which attention-averaging leaves
small next to x in LN(x+o) (measured 1.81e-3 rel err vs 1.66e-3
all-bf16). The FFN stays bf16: fp8 there costs ~1.9e-2 rel err — ff is
comparable to x1 in magnitude and nothing averages the noise away.
Weights are pre-scaled by 64 on the host so w*64 ~ N(0,1.3) stays in
fp8's normal range; the 1/64 is folded into the fused psum-evacuation
ops. K's bias is dropped outright: it shifts all of a query's scores
equally, which softmax cancels.

Attention: scores are computed TRANSPOSED (S^T[tk,q], lhsT=K^T chunk,
rhs=Q^T chunk) so exp(S^T) feeds the PV matmul directly as the moving
operand with token-major V as the stationary one. Softmax max-subtraction
is skipped (|s| < ~10, exp cannot overflow). The softmax denominator
comes FOR FREE from a ones-lane appended to the V stationary (65-wide
stationary -> row 64 of the PV psum is the exp row-sum); the reciprocal
row is broadcast across partitions with a contract-dim-1 fp32 matmul and
multiplied out on DVE. Exps are batched 4 token-tiles per Act
instruction to amortize the ~370ns Act fixed overhead.

w1 is streamed in per-column-block tiles into FFN1; w2 takes over V's
8MB SBUF slot (disjoint lifetimes), its load paced in chunks through
phases C/E so it never spikes the DMA engines. FFN2 runs kt-outer per
token-block (4 sequential psum accumulators) so each block's LayerNorm
hides under the next block's matmuls; bb2 is added by contract-dim-1
matmuls on the otherwise-idle PE. LayerNorm evacuations are fused:
psum-scale + residual-add + row-sum in one DVE op
(scalar_tensor_tensor with accum_out), variance via E[x^2]-m^2 on Act,
and the final affine+store is halved so the out DMA starts early."""

import numpy as np
import ml_dtypes

import concourse.bass as bass
import concourse.mybir as mybir
import concourse.tile as tile
from concourse.bass import ts, ds
from concourse.bass_utils import run_bass_kernel_spmd

BF16 = mybir.dt.bfloat16
FP32 = mybir.dt.float32
FP8 = mybir.dt.float8e4
F32R = mybir.dt.float32r
AF = mybir.ActivationFunctionType
ALU = mybir.AluOpType
DR = mybir.MatmulPerfMode.DoubleRow

S, B, D, H, DH, F = 2048, 2, 1024, 16, 64, 4096
NC = 8              # cores
CH = S // NC        # seq positions per core (256)
TQ = CH * B         # tokens per core (512)
P = 128
KT = D // P         # 8 k-tiles over D
MT = D // P         # 8 m-tiles over D
FT = F // P         # 32 tiles over F
TT = S // P         # 16 token-tiles per batch
LN_EPS = 1e-5
HP = H // 2         # 8 head-pairs
VW = DH + 1         # 65: V columns + ones lane
EG = 4              # token-tiles per batched exp
WS = 64.0           # host-side fp8 weight scale
WSI = 1.0 / WS


def _split_multiwaits(nc):
    # Walrus in this container encodes at most ONE sync-wait per instruction.
    # Tile's tail drain violates that; hoist extra waits onto fresh NoOps.
    for bb in nc.m.functions[0].blocks:
        new_insts = []
        for inst in bb.instructions:
            si = inst.sync_info
            if si is not None and si.on_wait and len(si.on_wait) > 1:
                waits = list(si.on_wait)
                for j, w in enumerate(waits[:-1]):
                    new_insts.append(mybir.InstNoOp(
                        name=f"{inst.name}-wsplit{j}", engine=inst.engine,
                        ins=[], outs=[],
                        sync_info=mybir.SyncInfo(on_wait=[w], on_update=[])))
                si.on_wait = [waits[-1]]
            new_insts.append(inst)
        bb.instructions = new_insts


def build_bass(split_waits=True, phases="ABCEF"):
    nc = bass.Bass(name="encoder_block", num_devices=NC, debug=False)

    # ---- I/O ----
    xT8 = nc.dram_tensor("xT8", (D, B, S), FP8, kind="ExternalInput")
    xTq8 = nc.dram_tensor("xTq8", (D, B, CH), FP8, kind="ExternalInput")
    xres = nc.dram_tensor("xres", (B, CH, D), FP32, kind="ExternalInput")
    wq8 = nc.dram_tensor("wq8", (D, D), FP8, kind="ExternalInput")
    wk8 = nc.dram_tensor("wk8", (D, D), FP8, kind="ExternalInput")
    wv8 = nc.dram_tensor("wv8", (D, D), FP8, kind="ExternalInput")
    wo8 = nc.dram_tensor("wo8", (D, D), FP8, kind="ExternalInput")
    w1s = nc.dram_tensor("w1s", (FT, P, KT, P), BF16, kind="ExternalInput")
    w2s = nc.dram_tensor("w2s", (FT // 2, P, 2, D), BF16,
                         kind="ExternalInput")
    identd = nc.dram_tensor("ident", (P, P), FP32, kind="ExternalInput")
    bqs = nc.dram_tensor("bqs", (D,), FP32, kind="ExternalInput")  # bq/8
    bb1 = nc.dram_tensor("bb1", (F,), FP32, kind="ExternalInput")  # *WS
    bb2r = nc.dram_tensor("bb2r", (1, D), FP32, kind="ExternalInput")  # *WS
    bv_rep = nc.dram_tensor("bv_rep", (P, D), FP32, kind="ExternalInput")
    g1_rep = nc.dram_tensor("g1_rep", (P, D), FP32, kind="ExternalInput")
    b1_rep = nc.dram_tensor("b1_rep", (P, D), FP32, kind="ExternalInput")
    g2_rep = nc.dram_tensor("g2_rep", (P, D), FP32, kind="ExternalInput")
    b2_rep = nc.dram_tensor("b2_rep", (P, D), FP32, kind="ExternalInput")
    out = nc.dram_tensor("out", (B, CH, D), FP32, kind="ExternalOutput")

    xT_t = xT8.rearrange("(kt p) b s -> p kt b s", p=P)
    xTq_t = xTq8.rearrange("(kt p) b s -> p kt b s", p=P)
    xres_t = xres.rearrange("b (tq p) d -> p b tq d", p=P)
    out_t = out.rearrange("b (tq p) d -> p b tq d", p=P)
    wq_t = wq8.rearrange("(kt p) n -> p kt n", p=P)
    wk_t = wk8.rearrange("(kt p) n -> p kt n", p=P)
    wv_t = wv8.rearrange("(kt p) n -> p kt n", p=P)
    wo_t = wo8.rearrange("(kt p) n -> p kt n", p=P)
    bqs_t = bqs.rearrange("(m p) -> p m", p=P)
    bb1_t = bb1.rearrange("(m p) -> p m", p=P)

    eps_sb = None

    def layer_norm_tail(pool, t1, ssum, g_sb, bt_sb, dst,
                        out_halves=None):
        """dst = LN(t1)*g + bt given t1 [P,D] fp32 and its row-sum ssum.

        Variance via E[x^2] - m^2: one Act Square pass over a scratch,
        tiny per-partition fixups, then a single fused (t1-m)*rstd DVE op.
        """
        sq = pool.tile([P, D], FP32, tag="ln_sq")
        ss = pool.tile([P, 1], FP32, tag="ln_ss")
        nc.scalar.activation(sq[:], t1[:], AF.Square, accum_out=ss[:])
        negmean = pool.tile([P, 1], FP32, tag="ln_negmean")
        nc.scalar.mul(negmean[:], ssum[:], -1.0 / D)
        m2 = pool.tile([P, 1], FP32, tag="ln_m2")
        nc.scalar.activation(m2[:], negmean[:], AF.Square)
        eb = pool.tile([P, 1], FP32, tag="ln_eb")
        nc.vector.tensor_scalar(eb[:], m2[:], -1.0, LN_EPS, ALU.mult,
                                ALU.add)
        st = pool.tile([P, 1], FP32, tag="ln_st")
        nc.scalar.activation(st[:], ss[:], AF.Sqrt, bias=eb[:],
                             scale=1.0 / D)
        rstd = pool.tile([P, 1], FP32, tag="ln_rstd")
        nc.vector.reciprocal(rstd[:], st[:])
        y = pool.tile([P, D], FP32, tag="ln_y")
        nc.vector.tensor_scalar(y[:], t1[:], negmean[:], rstd[:],
                                ALU.add, ALU.mult)
        if out_halves is None:
            yg = pool.tile([P, D], FP32, tag="ln_yg")
            nc.vector.tensor_tensor(yg[:], y[:], g_sb[:], ALU.mult)
            nc.vector.tensor_tensor(dst[:], yg[:], bt_sb[:], ALU.add)
        else:
            # halved final affine, each half DMA'd out immediately
            out_t, b, tq = out_halves
            for nbh in range(2):
                h = ds(nbh * (D // 2), D // 2)
                nc.vector.tensor_tensor(y[:, h], y[:, h], g_sb[:, h],
                                        ALU.mult)
                nc.vector.tensor_tensor(dst[:, h], y[:, h], bt_sb[:, h],
                                        ALU.add)
                nc.sync.dma_start(out_t[:, b, tq, h], dst[:, h])
        return y

    with tile.TileContext(nc) as tc:
        with (
            tc.tile_pool(name="persist", bufs=1) as pp,
            tc.tile_pool(name="dram", bufs=1, space="DRAM") as dpool,
        ):
            # alive for the whole kernel
            bqs_sb = pp.tile([P, MT], FP32, tag="bqs")
            bb1_sb = pp.tile([P, FT], FP32, tag="bb1")
            bb2r_sb = pp.tile([1, D], FP32, tag="bb2r")
            ones_row = pp.tile([1, P], FP32, tag="ones_row")

            kdram = dpool.tile([HP, P, B, S], FP8)               # K^T spill

            eps_sb = pp.tile([P, 1], FP32, tag="eps")
            nc.vector.memset(eps_sb[:], LN_EPS)
            nc.vector.memset(ones_row[:], 1.0)
            nc.gpsimd.dma_start(bqs_sb[:], bqs_t)
            nc.gpsimd.dma_start(bb1_sb[:], bb1_t)
            nc.gpsimd.dma_start(bb2r_sb[:], bb2r[:])

            with tc.tile_pool(name="x1p", bufs=1) as x1p:
                # alive A..F
                x1_sb = x1p.tile([P, B, B, D], FP32, tag="x1")
                x1T_sb = x1p.tile([P, KT, B, CH], BF16, tag="x1T")

                with tc.tile_pool(name="bigp", bufs=1) as bigp:
                  # one big slot reused across phases: V (A..B), then w2 (C..F)
                  v_sb = bigp.tile([P, B, TT, H, VW], BF16, tag="big")
                  with tc.tile_pool(name="otx", bufs=1) as otx:
                    # alive A..C (1 MB)
                    oT_sb = otx.tile([P, MT, B, CH], FP8, tag="oT")

                    with tc.tile_pool(name="vq", bufs=1) as vq:
                        qT_sb = vq.tile([P, MT, B, CH], BF16, tag="qT")
                        k0_sb = vq.tile([P, B, S], FP8, tag="k0")
                        bvr_sb = vq.tile([P, D], FP32, tag="bvr")
                        ones_col = vq.tile([1, DH], F32R, tag="ones_col")
                        ones_f32 = vq.tile([1, DH], FP32, tag="ones_f32")
                        nc.gpsimd.dma_start(bvr_sb[:], bv_rep[:])
                        nc.vector.memset(ones_f32[:], 1.0)
                        with nc.allow_low_precision(
                                reason="f32r ones for 1-cyc/row broadcast"):
                            nc.vector.tensor_copy(ones_col[:], ones_f32[:])
                        # ones lane for the fused softmax row-sum
                        # (on the otherwise-idle gpsimd engine)
                        nc.gpsimd.memset(
                            v_sb[:, :, :, :, ds(DH, 1)], 1.0)

                        # ===== Phase A: projections (K^T, V, Q^T) =====
                        # aout outlives A: its ksb tiles' last readers are
                        # kdram-write DMAs that drain late; keeping the
                        # pool open stops B's pools from WAR-waiting on
                        # that space.
                        apool_cm = tc.tile_pool(name="aout", bufs=6)
                        apool = apool_cm.__enter__()
                        with (
                            tc.tile_pool(name="wqp", bufs=1) as wqpool,
                            tc.tile_pool(name="wqkv", bufs=2) as wpool,
                            tc.tile_pool(name="xt", bufs=3) as xpool,
                            tc.tile_pool(name="psA", bufs=5,
                                         space="PSUM") as psA,
                        ):
                            # initial loads fan out over the DMA queues
                            wq_sb = wqpool.tile([P, KT, D], FP8, tag="wq")
                            nc.gpsimd.dma_start(wq_sb[:], wq_t)
                            wk_sb = wpool.tile([P, KT, D], FP8, tag="w")
                            nc.sync.dma_start(wk_sb[:], wk_t)
                            wv_sb = wpool.tile([P, KT, D], FP8, tag="w")
                            nc.scalar.dma_start(wv_sb[:], wv_t)
                            xtq_sb = xpool.tile([P, KT, B, CH], FP8,
                                                tag="xtq")
                            nc.gpsimd.dma_start(xtq_sb[:], xTq_t)

                            # Q^T first (only needs wq+xtq): its DVE
                            # evacs land early so phase B's first scores
                            # don't wait on A's whole DVE queue, and the
                            # matmuls fill the initial weight-DMA window.
                            for b in range(B):
                                for m in range(MT):
                                    ps = psA.tile([P, CH], FP32, tag="psq",
                                                  bufs=3)
                                    for kt in range(0, KT, 2):
                                        nc.tensor.matmul(
                                            ps[:],
                                            wq_sb[:, ds(kt, 2), ts(m, P)],
                                            xtq_sb[:, ds(kt, 2), b, :],
                                            start=(kt == 0),
                                            stop=(kt == KT - 2),
                                            perf_mode=DR)
                                    nc.vector.tensor_scalar(
                                        qT_sb[:, m, b, :], ps[:],
                                        0.125 * WSI,
                                        bqs_sb[:, ds(m, 1)],
                                        ALU.mult, ALU.add)

                            SQ = S // 4  # 512-token stream chunks
                            for b in range(B):
                                for sh in range(4):
                                    xth = xpool.tile([P, KT, SQ], FP8,
                                                     tag="xth")
                                    (nc.scalar if (b + sh) == 0 else
                                     nc.sync).dma_start(
                                        xth[:],
                                        xT_t[:, :, b, ds(sh * SQ, SQ)])
                                    # K^T and V interleaved so the Act
                                    # (K-evac) and DVE (V-evac) engines
                                    # alternate and psum banks recycle
                                    # without bursty evac lag.
                                    # NOTE: bk is dropped on purpose:
                                    # K's bias adds q.bk to every score
                                    # of a query, which softmax cancels.
                                    for m in range(MT):
                                        ps = psA.tile([P, 512], FP32,
                                                      tag="psa")
                                        for kt in range(0, KT, 2):
                                            nc.tensor.matmul(
                                                ps[:],
                                                wk_sb[:, ds(kt, 2),
                                                      ts(m, P)],
                                                xth[:, ds(kt, 2), :],
                                                start=(kt == 0),
                                                stop=(kt == KT - 2),
                                                perf_mode=DR)
                                        if m == 0:
                                            nc.scalar.activation(
                                                k0_sb[:, b, ds(sh * SQ, SQ)],
                                                ps[:], AF.Copy,
                                                bias=0.0, scale=WSI)
                                        else:
                                            ksb = apool.tile([P, 512], FP8,
                                                             tag="ksb")
                                            nc.scalar.activation(
                                                ksb[:], ps[:], AF.Copy,
                                                bias=0.0, scale=WSI)
                                            nc.gpsimd.dma_start(
                                                kdram[m, :, b,
                                                      ds(sh * SQ, SQ)],
                                                ksb[:])
                                        # V (token-major, 65-lane layout)
                                        tl, nb = divmod(m, D // 512)
                                        tt = sh * (SQ // P) + tl
                                        ps = psA.tile([P, 512], FP32,
                                                      tag="psa")
                                        for kt in range(0, KT, 2):
                                            nc.tensor.matmul(
                                                ps[:],
                                                xth[:, ds(kt, 2),
                                                    ts(tl, P)],
                                                wv_sb[:, ds(kt, 2),
                                                      ts(nb, 512)],
                                                start=(kt == 0),
                                                stop=(kt == KT - 2),
                                                perf_mode=DR)
                                        nc.vector.scalar_tensor_tensor(
                                            v_sb[:, b, tt,
                                                 ds(nb * 8, 8),
                                                 ds(0, DH)],
                                            ps[:], WSI,
                                            bvr_sb[:, ts(nb, 512)],
                                            ALU.mult, ALU.add)

                        # ===== Phase B: attention =====
                        # prefetch phase-C weights while attention runs
                        with tc.tile_pool(name="wo_p", bufs=1) as wopool:
                            wo_sb = wopool.tile([P, KT, D], FP8, tag="wo")
                            nc.scalar.dma_start(wo_sb[:], wo_t)
                            ident = wopool.tile([P, P], FP32, tag="ident")
                            nc.scalar.dma_start(ident[:], identd[:])
                            g1r_sb = wopool.tile([P, D], FP32, tag="g1r")
                            b1r_sb = wopool.tile([P, D], FP32, tag="b1r")
                            nc.gpsimd.dma_start(g1r_sb[:], g1_rep[:])
                            nc.gpsimd.dma_start(b1r_sb[:], b1_rep[:])

                            if "B" not in phases:
                                nc.vector.memset(oT_sb[:], 0.001)
                            with (
                                tc.tile_pool(name="kpair", bufs=3) as kpool,
                                tc.tile_pool(name="expst", bufs=3) as epool,
                                tc.tile_pool(name="battn", bufs=4) as bpool,
                                tc.tile_pool(name="psS", bufs=2,
                                             space="PSUM") as psS,
                                tc.tile_pool(name="psO", bufs=2,
                                             space="PSUM") as psO,
                                tc.tile_pool(name="psR", bufs=2,
                                             space="PSUM") as psR,
                            ):
                                hpb = [(b, hp) for b in range(B)
                                       for hp in range(HP)]
                                for b, hp in (hpb if "B" in phases else ()):
                                    if True:
                                        if hp == 0:
                                            kpair = k0_sb[:, b, :]
                                        else:
                                            kp_t = kpool.tile([P, S], FP8,
                                                              tag="kpair")
                                            nc.sync.dma_start(
                                                kp_t[:], kdram[hp, :, b, :])
                                            kpair = kp_t
                                        for h01 in range(2):
                                            po = h01 * DH
                                            h = hp * 2 + h01
                                            expst = epool.tile(
                                                [P, TT, CH], BF16,
                                                tag="expst")
                                            for g in range(TT // EG):
                                                pss = psS.tile(
                                                    [P, EG, CH], FP32,
                                                    tag="pss")
                                                for j in range(EG):
                                                    tt = g * EG + j
                                                    nc.tensor.matmul(
                                                        pss[:, j, :],
                                                        kpair[ds(po, DH),
                                                              ts(tt, P)],
                                                        qT_sb[ds(po, DH),
                                                              hp, b, :])
                                                nc.scalar.activation(
                                                    expst[:, ds(g * EG, EG),
                                                          :],
                                                    pss[:], AF.Exp)
                                            po_ps = psO.tile([VW, CH], FP32,
                                                             tag="pso")
                                            for tt in range(TT):
                                                nc.tensor.matmul(
                                                    po_ps[:],
                                                    v_sb[:, b, tt, h, :],
                                                    expst[:, tt, :],
                                                    start=(tt == 0),
                                                    stop=(tt == TT - 1))
                                            # softmax denominators sit in
                                            # row 64 (the ones lane)
                                            rsum = bpool.tile([1, CH], FP32,
                                                              tag="rsum")
                                            nc.vector.tensor_copy(
                                                rsum[:],
                                                po_ps[ds(DH, 1), :])
                                            rec = bpool.tile([1, CH], F32R,
                                                             tag="rec")
                                            with nc.allow_low_precision(
                                                    reason="f32r recip for "
                                                    "1-cyc/row broadcast"):
                                                nc.vector.reciprocal(
                                                    rec[:], rsum[:])
                                            # broadcast across partitions:
                                            # ones[1,64]^T @ rec[1,CH]
                                            recb = psR.tile([DH, CH], FP32,
                                                            tag="recb")
                                            nc.tensor.matmul(
                                                recb[:], ones_col[:],
                                                rec[:],
                                                start=True, stop=True)
                                            recb_sb = bpool.tile(
                                                [DH, CH], FP32, tag="recb_sb")
                                            nc.vector.tensor_copy(
                                                recb_sb[:], recb[:])
                                            with nc.allow_low_precision(
                                                    reason="oT stored fp8; "
                                                    "noise hidden by "
                                                    "residual LN"):
                                                nc.vector.tensor_tensor(
                                                    oT_sb[ds(po, DH),
                                                          hp, b, :],
                                                    po_ps[ds(0, DH), :],
                                                    recb_sb[:], ALU.mult)

                    # ===== Phase C: O-proj + residual + LN1 (+ x1^T) =====
                            # w2 takes over V's SBUF slot; chunks are
                            # paced through C and E to avoid a DMA spike
                            w2r_sb = bigp.tile([P, FT // 2, 2, D + 16],
                                               BF16, tag="big")
                            if "C" not in phases:
                                nc.vector.memset(x1_sb[:], 0.001)
                                nc.vector.memset(x1T_sb[:], 0.001)
                            with (
                                tc.tile_pool(name="cscr", bufs=2) as cpool,
                                tc.tile_pool(name="psC", bufs=2,
                                             space="PSUM") as psC,
                                tc.tile_pool(name="psD", bufs=2,
                                             space="PSUM") as psD,
                            ):
                                for b in range(B if "C" in phases else 0):
                                    for tq in range(B):
                                        kp = b * B + tq
                                        nc.gpsimd.dma_start(
                                            w2r_sb[:, kp, :, ds(0, D)],
                                            w2s[kp])
                                        ps = psC.tile([P, D], FP32, tag="psc")
                                        for nb in range(D // 512):
                                            for kt in range(0, KT, 2):
                                                nc.tensor.matmul(
                                                    ps[:, ts(nb, 512)],
                                                    oT_sb[:, ds(kt, 2), b,
                                                          ts(tq, P)],
                                                    wo_sb[:, ds(kt, 2),
                                                          ts(nb, 512)],
                                                    start=(kt == 0),
                                                    stop=(kt == KT - 2),
                                                    perf_mode=DR)
                                        xres_sb = cpool.tile([P, D], FP32,
                                                             tag="xres")
                                        nc.sync.dma_start(
                                            xres_sb[:], xres_t[:, b, tq, :])
                                        # fused: t1 = ps + (x + bo),
                                        # row-sum for the LN mean
                                        t1 = cpool.tile([P, D], FP32,
                                                        tag="c_t1")
                                        ssum = cpool.tile([P, 1], FP32,
                                                          tag="c_ssum")
                                        nc.vector.scalar_tensor_tensor(
                                            t1[:], ps[:], WSI, xres_sb[:],
                                            ALU.mult, ALU.add,
                                            accum_out=ssum[:])
                                        y_t = layer_norm_tail(
                                            cpool, t1, ssum, g1r_sb, b1r_sb,
                                            x1_sb[:, b, tq, :])
                                        # transposes take the PRE-affine y:
                                        # g1 is folded into W1 host-side
                                        # (bb1' = bb1 + b1@W1), so FFN1 is
                                        # exact while the affine x1 (for
                                        # the F residual) finishes in
                                        # parallel.
                                        for kd in range(KT):
                                            pt = psD.tile([P, P], FP32,
                                                          tag="psd")
                                            nc.tensor.transpose(
                                                pt[:],
                                                y_t[:, ts(kd, P)],
                                                ident[:])
                                            nc.scalar.copy(
                                                x1T_sb[:, kd, b, ts(tq, P)],
                                                pt[:])
                        apool_cm.__exit__(None, None, None)

                  # ===== Phase E: FFN1  hT = relu(x1@W1+bb1)^T =====
                  with tc.tile_pool(name="hT", bufs=1) as hpool:
                      hT_sb = hpool.tile([P, FT, TQ], BF16, tag="hT")
                      with (
                          tc.tile_pool(name="w1_p", bufs=4) as w1pool,
                          tc.tile_pool(name="psE", bufs=4,
                                       space="PSUM") as psE,
                      ):
                          if "E" not in phases:
                              nc.vector.memset(hT_sb[:], 0.001)
                          for mh in range(FT if "E" in phases else 0):
                              if mh % 2 == 0 and 4 + mh // 2 < FT // 2:
                                  kp = 4 + mh // 2
                                  nc.gpsimd.dma_start(
                                      w2r_sb[:, kp, :, ds(0, D)],
                                      w2s[kp])
                              w1t = w1pool.tile([P, KT, P], BF16, tag="w1t")
                              nc.sync.dma_start(w1t[:], w1s[mh])
                              ps = psE.tile([P, TQ], FP32, tag="pse")
                              for kt in range(KT):
                                  nc.tensor.matmul(
                                      ps[:], w1t[:, kt, :],
                                      x1T_sb[:, kt, :, :],
                                      start=(kt == 0), stop=(kt == KT - 1))
                              nc.scalar.activation(
                                  hT_sb[:, mh, :], ps[:], AF.Relu,
                                  bias=bb1_sb[:, ds(mh, 1)])

                      # ===== Phase F: FFN2 + residual + LN2 -> out =====
                      # kt-outer, two token-halves; w2 streamed per kt-pair
                      with (
                          tc.tile_pool(name="fscr", bufs=2) as fpool,
                          tc.tile_pool(name="fbias", bufs=1) as fbp,
                          tc.tile_pool(name="psF", bufs=2, space="PSUM") as psF,
                      ):
                          g2r_sb = fbp.tile([P, D], FP32, tag="g2r")
                          b2r_sb = fbp.tile([P, D], FP32, tag="b2r")
                          nc.gpsimd.dma_start(g2r_sb[:], g2_rep[:])
                          nc.gpsimd.dma_start(b2r_sb[:], b2_rep[:])
                          if "F" not in phases:
                              for b in range(B):
                                  for tq in range(B):
                                      dummy = fpool.tile([P, D], FP32,
                                                         tag="f_out")
                                      nc.vector.memset(dummy[:], 0.5)
                                      nc.sync.dma_start(out_t[:, b, tq, :],
                                                        dummy[:])
                          for c in range(B * B if "F" in phases else 0):
                              ps_f = psF.tile([P, D], FP32, tag=f"psf{c % 2}")
                              for kp in range(FT // 2):
                                  for kj in range(2):
                                      for nb in range(D // 512):
                                          nc.tensor.matmul(
                                              ps_f[:, ts(nb, 512)],
                                              hT_sb[:, 2 * kp + kj,
                                                    ts(c, P)],
                                              w2r_sb[:, kp, kj, ts(nb, 512)],
                                              start=(kp == 0 and kj == 0),
                                              stop=False)
                              # bb2 via contract-1 matmuls on the idle
                              # PE; closes the accumulation group
                              for nb in range(D // 512):
                                  nc.tensor.matmul(
                                      ps_f[:, ts(nb, 512)],
                                      ones_row[:],
                                      bb2r_sb[:, ts(nb, 512)],
                                      start=False,
                                      stop=(nb == D // 512 - 1))
                              if True:
                                  b, tq = divmod(c, B)
                                  t1 = fpool.tile([P, D], FP32, tag="f_t1")
                                  ssum = fpool.tile([P, 1], FP32,
                                                    tag="f_ssum")
                                  nc.vector.scalar_tensor_tensor(
                                      t1[:], ps_f[:], 1.0,
                                      x1_sb[:, b, tq, :],
                                      ALU.mult, ALU.add, accum_out=ssum[:])
                                  o_sb = fpool.tile([P, D], FP32, tag="f_out")
                                  layer_norm_tail(fpool, t1, ssum,
                                                  g2r_sb, b2r_sb, o_sb,
                                                  out_halves=(out_t, b, tq))

    if split_waits:
        _split_multiwaits(nc)
    return nc


_NC_CACHE = None


def _get_bass():
    global _NC_CACHE
    if _NC_CACHE is None:
        _NC_CACHE = build_bass()
    return _NC_CACHE


def make_in_maps(x, Wq, bq, Wk, bk, Wv, bv, Wo, bo, g1, b1, W1, bb1, W2, bb2,
                 g2, b2):
    bf = ml_dtypes.bfloat16
    f8 = ml_dtypes.float8_e4m3
    x = np.asarray(x, np.float32)
    xT = np.ascontiguousarray(x.transpose(2, 1, 0))              # [D,B,S]
    W1 = np.asarray(W1, np.float32)
    W2 = np.asarray(W2, np.float32)
    g1f = np.asarray(g1, np.float32)
    b1f = np.asarray(b1, np.float32)
    bb1 = np.asarray(bb1, np.float32) + b1f @ W1
    W1 = g1f[:, None] * W1
    # w1s[mh] = W1[:, mh*128:(mh+1)*128] rearranged [(kt p), n] -> [p kt n]
    w1s = np.ascontiguousarray(
        W1.reshape(KT, P, FT, P).transpose(2, 1, 0, 3)).astype(bf)
    # w2s[kp] = W2[kp*256:(kp+1)*256, :] as [P, 2, D]
    w2s = np.ascontiguousarray(
        W2.reshape(FT // 2, 2, P, D).transpose(0, 2, 1, 3)).astype(bf)
    shared = {
        "xT8": xT.astype(f8),
        "wq8": (np.asarray(Wq, np.float32) * WS).astype(f8),
        "wk8": (np.asarray(Wk, np.float32) * WS).astype(f8),
        "wv8": (np.asarray(Wv, np.float32) * WS).astype(f8),
        "wo8": (np.asarray(Wo, np.float32) * WS).astype(f8),
        "w1s": w1s,
        "w2s": w2s,
        "ident": np.eye(P, dtype=np.float32),
        "bqs": (np.asarray(bq, np.float32) / 8.0),
        "bb1": bb1,
        "bb2r": np.asarray(bb2, np.float32).reshape(1, D),
        "bv_rep": np.tile(np.asarray(bv, np.float32), (P, 1)),
        "g1_rep": np.tile(np.asarray(g1, np.float32), (P, 1)),
        "b1_rep": np.tile(np.asarray(b1, np.float32), (P, 1)),
        "g2_rep": np.tile(np.asarray(g2, np.float32), (P, 1)),
        "b2_rep": np.tile(np.asarray(b2, np.float32), (P, 1)),
    }
    xf8 = xT.astype(f8)
    xpbo = x + np.asarray(bo, np.float32)       # fold bo into the residual
    in_maps = []
    for c in range(NC):
        sl = slice(c * CH, (c + 1) * CH)
        m = dict(shared)
        m["xTq8"] = np.ascontiguousarray(xf8[:, :, sl])
        m["xres"] = np.ascontiguousarray(
            xpbo[sl].transpose(1, 0, 2))           # [B, CH, D]
        in_maps.append(m)
    return in_maps


def assemble(results):
    out = np.empty((S, B, D), np.float32)
    for c, r in enumerate(results):
        out[c * CH:(c + 1) * CH] = r["out"].transpose(1, 0, 2)
    return out


def kernel(**inputs) -> np.ndarray:
    nc = _get_bass()
    in_maps = make_in_maps(**inputs)
    res = run_bass_kernel_spmd(nc, in_maps, core_ids=list(range(NC)))
    return assemble(res.results)



# revision 2
# speedup vs baseline: 49.6755x; 49.6755x over previous
"""Trainium2 Bass kernel for nn_EncoderBlock (dense transformer block).

Reference computation (fp32, S=2048 B=2 D=1024 H=16 dh=64 F=4096):
    q,k,v = x@Wq+bq, x@Wk+bk, x@Wv+bv          (per-head split, dh=64)
    attn  = softmax(q k^T / sqrt(dh)) v         (full S x S scores)
    o     = attn-merge @ Wo + bo
    x1    = LN(x + o; g1,b1)
    out   = LN(x1 + relu(x1@W1+bb1)@W2+bb2; g2,b2)

Sharding: sequence-parallel over 8 cores. Each core owns 256 seq positions
(x 2 batches = 512 tokens) end-to-end; K/V are computed redundantly on every
core (an on-chip collective costs more than the recompute at this size).

Precision: the Q/K/V projections and the O-projection run in fp8-e4m3
with DoubleRow perf mode (2 k-tiles per instruction, 0.5 cycles/row =>
4x bf16 throughput). QKV noise washes out in the softmax average over
2048 keys; O-proj noise rides on o, which attention-averaging leaves
small next to x in LN(x+o) (measured 1.81e-3 rel err vs 1.66e-3
all-bf16). The FFN stays bf16: fp8 there costs ~1.9e-2 rel err — ff is
comparable to x1 in magnitude and nothing averages the noise away.
Weights are pre-scaled by 64 on the host so w*64 ~ N(0,1.3) stays in
fp8's normal range; the 1/64 is folded into the fused psum-evacuation
ops. K's bias is dropped outright: it shifts all of a query's scores
equally, which softmax cancels.

Attention: scores are computed TRANSPOSED (S^T[tk,q], lhsT=K^T chunk,
rhs=Q^T chunk) so exp(S^T) feeds the PV matmul directly as the moving
operand with token-major V as the stationary one. Softmax max-subtraction
is skipped (|s| < ~10, exp cannot overflow). The softmax denominator
comes FOR FREE from a ones-lane appended to the V stationary (65-wide
stationary -> row 64 of the PV psum is the exp row-sum); the reciprocal
row is broadcast across partitions with a contract-dim-1 fp32 matmul and
multiplied out on DVE. Exps are batched 4 token-tiles per Act
instruction to amortize the ~370ns Act fixed overhead.

w1 is streamed in per-column-block tiles into FFN1; w2 takes over V's
8MB SBUF slot (disjoint lifetimes), its load paced in chunks through
phases C/E so it never spikes the DMA engines. FFN2 runs kt-outer per
token-block (4 sequential psum accumulators) so each block's LayerNorm
hides under the next block's matmuls; bb2 is added by contract-dim-1
matmuls on the otherwise-idle PE. LayerNorm evacuations are fused:
psum-scale + residual-add + row-sum in one DVE op
(scalar_tensor_tensor with accum_out), variance via E[x^2]-m^2 on Act,
and the final affine+store is halved so the out DMA starts early."""

import numpy as np
import ml_dtypes

import concourse.bass as bass
import concourse.mybir as mybir
import concourse.tile as tile
from concourse.bass import ts, ds
from concourse.bass_utils import run_bass_kernel_spmd

BF16 = mybir.dt.bfloat16
FP32 = mybir.dt.float32
FP8 = mybir.dt.float8e4
F32R = mybir.dt.float32r
AF = mybir.ActivationFunctionType
ALU = mybir.AluOpType
DR = mybir.MatmulPerfMode.DoubleRow

S, B, D, H, DH, F = 2048, 2, 1024, 16, 64, 4096
NC = 8              # cores
CH = S // NC        # seq positions per core (256)
TQ = CH * B         # tokens per core (512)
P = 128
KT = D // P         # 8 k-tiles over D
MT = D // P         # 8 m-tiles over D
FT = F // P         # 32 tiles over F
TT = S // P         # 16 token-tiles per batch
LN_EPS = 1e-5
HP = H // 2         # 8 head-pairs
VW = DH + 1         # 65: V columns + ones lane
EG = 4              # token-tiles per batched exp
WS = 64.0           # host-side fp8 weight scale
WSI = 1.0 / WS


def _split_multiwaits(nc):
    # Walrus in this container encodes at most ONE sync-wait per instruction.
    # Tile's tail drain violates that; hoist extra waits onto fresh NoOps.
    for bb in nc.m.functions[0].blocks:
        new_insts = []
        for inst in bb.instructions:
            si = inst.sync_info
            if si is not None and si.on_wait and len(si.on_wait) > 1:
                waits = list(si.on_wait)
                for j, w in enumerate(waits[:-1]):
                    new_insts.append(mybir.InstNoOp(
                        name=f"{inst.name}-wsplit{j}", engine=inst.engine,
                        ins=[], outs=[],
                        sync_info=mybir.SyncInfo(on_wait=[w], on_update=[])))
                si.on_wait = [waits[-1]]
            new_insts.append(inst)
        bb.instructions = new_insts


def build_bass(split_waits=True, phases="ABCEF"):
    nc = bass.Bass(name="encoder_block", num_devices=NC, debug=False)

    # ---- I/O ----
    xT8 = nc.dram_tensor("xT8", (D, B, S), FP8, kind="ExternalInput")
    xTq8 = nc.dram_tensor("xTq8", (D, B, CH), FP8, kind="ExternalInput")
    xres = nc.dram_tensor("xres", (B, CH, D), FP32, kind="ExternalInput")
    wq8 = nc.dram_tensor("wq8", (D, D), FP8, kind="ExternalInput")
    wk8 = nc.dram_tensor("wk8", (D, D), FP8, kind="ExternalInput")
    wv8 = nc.dram_tensor("wv8", (D, D), FP8, kind="ExternalInput")
    wo8 = nc.dram_tensor("wo8", (D, D), FP8, kind="ExternalInput")
    w1s = nc.dram_tensor("w1s", (FT, P, KT, P), BF16, kind="ExternalInput")
    w2s = nc.dram_tensor("w2s", (FT // 2, P, 2, D), BF16,
                         kind="ExternalInput")
    identd = nc.dram_tensor("ident", (P, P), FP32, kind="ExternalInput")
    bqs = nc.dram_tensor("bqs", (D,), FP32, kind="ExternalInput")  # bq/8
    bb1 = nc.dram_tensor("bb1", (F,), FP32, kind="ExternalInput")  # *WS
    bb2r = nc.dram_tensor("bb2r", (1, D), FP32, kind="ExternalInput")  # *WS
    bv_rep = nc.dram_tensor("bv_rep", (P, D), FP32, kind="ExternalInput")
    g1_rep = nc.dram_tensor("g1_rep", (P, D), FP32, kind="ExternalInput")
    b1_rep = nc.dram_tensor("b1_rep", (P, D), FP32, kind="ExternalInput")
    g2_rep = nc.dram_tensor("g2_rep", (P, D), FP32, kind="ExternalInput")
    b2_rep = nc.dram_tensor("b2_rep", (P, D), FP32, kind="ExternalInput")
    out = nc.dram_tensor("out", (B, CH, D), FP32, kind="ExternalOutput")

    xT_t = xT8.rearrange("(kt p) b s -> p kt b s", p=P)
    xTq_t = xTq8.rearrange("(kt p) b s -> p kt b s", p=P)
    xres_t = xres.rearrange("b (tq p) d -> p b tq d", p=P)
    out_t = out.rearrange("b (tq p) d -> p b tq d", p=P)
    wq_t = wq8.rearrange("(kt p) n -> p kt n", p=P)
    wk_t = wk8.rearrange("(kt p) n -> p kt n", p=P)
    wv_t = wv8.rearrange("(kt p) n -> p kt n", p=P)
    wo_t = wo8.rearrange("(kt p) n -> p kt n", p=P)
    bqs_t = bqs.rearrange("(m p) -> p m", p=P)
    bb1_t = bb1.rearrange("(m p) -> p m", p=P)

    eps_sb = None

    def layer_norm_tail(pool, t1, ssum, g_sb, bt_sb, dst,
                        out_halves=None):
        """dst = LN(t1)*g + bt given t1 [P,D] fp32 and its row-sum ssum.

        Variance via E[x^2] - m^2: one Act Square pass over a scratch,
        tiny per-partition fixups, then a single fused (t1-m)*rstd DVE op.
        """
        sq = pool.tile([P, D], FP32, tag="ln_sq")
        ss = pool.tile([P, 1], FP32, tag="ln_ss")
        nc.scalar.activation(sq[:], t1[:], AF.Square, accum_out=ss[:])
        negmean = pool.tile([P, 1], FP32, tag="ln_negmean")
        nc.scalar.mul(negmean[:], ssum[:], -1.0 / D)
        m2 = pool.tile([P, 1], FP32, tag="ln_m2")
        nc.scalar.activation(m2[:], negmean[:], AF.Square)
        eb = pool.tile([P, 1], FP32, tag="ln_eb")
        nc.vector.tensor_scalar(eb[:], m2[:], -1.0, LN_EPS, ALU.mult,
                                ALU.add)
        st = pool.tile([P, 1], FP32, tag="ln_st")
        nc.scalar.activation(st[:], ss[:], AF.Sqrt, bias=eb[:],
                             scale=1.0 / D)
        rstd = pool.tile([P, 1], FP32, tag="ln_rstd")
        nc.vector.reciprocal(rstd[:], st[:])
        y = pool.tile([P, D], FP32, tag="ln_y")
        nc.vector.tensor_scalar(y[:], t1[:], negmean[:], rstd[:],
                                ALU.add, ALU.mult)
        if out_halves is None:
            yg = pool.tile([P, D], FP32, tag="ln_yg")
            nc.vector.tensor_tensor(yg[:], y[:], g_sb[:], ALU.mult)
            nc.vector.tensor_tensor(dst[:], yg[:], bt_sb[:], ALU.add)
        else:
            # halved final affine, each half DMA'd out immediately
            out_t, b, tq = out_halves
            for nbh in range(2):
                h = ds(nbh * (D // 2), D // 2)
                nc.vector.tensor_tensor(y[:, h], y[:, h], g_sb[:, h],
                                        ALU.mult)
                nc.vector.tensor_tensor(dst[:, h], y[:, h], bt_sb[:, h],
                                        ALU.add)
                nc.sync.dma_start(out_t[:, b, tq, h], dst[:, h])
        return y

    with tile.TileContext(nc) as tc:
        with (
            tc.tile_pool(name="persist", bufs=1) as pp,
            tc.tile_pool(name="dram", bufs=1, space="DRAM") as dpool,
        ):
            # alive for the whole kernel
            bqs_sb = pp.tile([P, MT], FP32, tag="bqs")
            bb1_sb = pp.tile([P, FT], FP32, tag="bb1")
            bb2r_sb = pp.tile([1, D], FP32, tag="bb2r")
            ones_row = pp.tile([1, P], FP32, tag="ones_row")

            kdram = dpool.tile([HP, P, B, S], FP8)               # K^T spill

            eps_sb = pp.tile([P, 1], FP32, tag="eps")
            nc.vector.memset(eps_sb[:], LN_EPS)
            nc.vector.memset(ones_row[:], 1.0)
            nc.gpsimd.dma_start(bqs_sb[:], bqs_t)
            nc.gpsimd.dma_start(bb1_sb[:], bb1_t)
            nc.gpsimd.dma_start(bb2r_sb[:], bb2r[:])

            with tc.tile_pool(name="x1p", bufs=1) as x1p:
                # alive A..F
                x1_sb = x1p.tile([P, B, B, D], FP32, tag="x1")
                x1T_sb = x1p.tile([P, KT, B, CH], BF16, tag="x1T")

                with tc.tile_pool(name="bigp", bufs=1) as bigp:
                  # one big slot reused across phases: V (A..B), then w2 (C..F)
                  v_sb = bigp.tile([P, B, TT, H, VW], BF16, tag="big")
                  with tc.tile_pool(name="otx", bufs=1) as otx:
                    # alive A..C (1 MB)
                    oT_sb = otx.tile([P, MT, B, CH], FP8, tag="oT")

                    with tc.tile_pool(name="vq", bufs=1) as vq:
                        qT_sb = vq.tile([P, MT, B, CH], BF16, tag="qT")
                        k0_sb = vq.tile([P, B, S], FP8, tag="k0")
                        bvr_sb = vq.tile([P, D], FP32, tag="bvr")
                        ones_col = vq.tile([1, DH], F32R, tag="ones_col")
                        ones_f32 = vq.tile([1, DH], FP32, tag="ones_f32")
                        nc.gpsimd.dma_start(bvr_sb[:], bv_rep[:])
                        nc.vector.memset(ones_f32[:], 1.0)
                        with nc.allow_low_precision(
                                reason="f32r ones for 1-cyc/row broadcast"):
                            nc.vector.tensor_copy(ones_col[:], ones_f32[:])
                        # ones lane for the fused softmax row-sum
                        # (on the otherwise-idle gpsimd engine)
                        nc.gpsimd.memset(
                            v_sb[:, :, :, :, ds(DH, 1)], 1.0)

                        # ===== Phase A: projections (K^T, V, Q^T) =====
                        # aout outlives A: its ksb tiles' last readers are
                        # kdram-write DMAs that drain late; keeping the
                        # pool open stops B's pools from WAR-waiting on
                        # that space.
                        apool_cm = tc.tile_pool(name="aout", bufs=6)
                        apool = apool_cm.__enter__()
                        with (
                            tc.tile_pool(name="wqp", bufs=1) as wqpool,
                            tc.tile_pool(name="wqkv", bufs=2) as wpool,
                            tc.tile_pool(name="xt", bufs=3) as xpool,
                            tc.tile_pool(name="psA", bufs=5,
                                         space="PSUM") as psA,
                        ):
                            # initial loads fan out over the DMA queues
                            wq_sb = wqpool.tile([P, KT, D], FP8, tag="wq")
                            nc.gpsimd.dma_start(wq_sb[:], wq_t)
                            wk_sb = wpool.tile([P, KT, D], FP8, tag="w")
                            nc.sync.dma_start(wk_sb[:], wk_t)
                            wv_sb = wpool.tile([P, KT, D], FP8, tag="w")
                            nc.scalar.dma_start(wv_sb[:], wv_t)
                            xtq_sb = xpool.tile([P, KT, B, CH], FP8,
                                                tag="xtq")
                            nc.gpsimd.dma_start(xtq_sb[:], xTq_t)

                            # Q^T first (only needs wq+xtq): its DVE
                            # evacs land early so phase B's first scores
                            # don't wait on A's whole DVE queue, and the
                            # matmuls fill the initial weight-DMA window.
                            for b in range(B):
                                for m in range(MT):
                                    ps = psA.tile([P, CH], FP32, tag="psq",
                                                  bufs=3)
                                    for kt in range(0, KT, 2):
                                        nc.tensor.matmul(
                                            ps[:],
                                            wq_sb[:, ds(kt, 2), ts(m, P)],
                                            xtq_sb[:, ds(kt, 2), b, :],
                                            start=(kt == 0),
                                            stop=(kt == KT - 2),
                                            perf_mode=DR)
                                    nc.vector.tensor_scalar(
                                        qT_sb[:, m, b, :], ps[:],
                                        0.125 * WSI,
                                        bqs_sb[:, ds(m, 1)],
                                        ALU.mult, ALU.add)

                            SQ = S // 4  # 512-token stream chunks
                            for b in range(B):
                                for sh in range(4):
                                    xth = xpool.tile([P, KT, SQ], FP8,
                                                     tag="xth")
                                    (nc.scalar if (b + sh) == 0 else
                                     nc.sync).dma_start(
                                        xth[:],
                                        xT_t[:, :, b, ds(sh * SQ, SQ)])
                                    # K^T and V interleaved so the Act
                                    # (K-evac) and DVE (V-evac) engines
                                    # alternate and psum banks recycle
                                    # without bursty evac lag.
                                    # NOTE: bk is dropped on purpose:
                                    # K's bias adds q.bk to every score
                                    # of a query, which softmax cancels.
                                    for m in range(MT):
                                        ps = psA.tile([P, 512], FP32,
                                                      tag="psa")
                                        for kt in range(0, KT, 2):
                                            nc.tensor.matmul(
                                                ps[:],
                                                wk_sb[:, ds(kt, 2),
                                                      ts(m, P)],
                                                xth[:, ds(kt, 2), :],
                                                start=(kt == 0),
                                                stop=(kt == KT - 2),
                                                perf_mode=DR)
                                        if m == 0:
                                            nc.scalar.activation(
                                                k0_sb[:, b, ds(sh * SQ, SQ)],
                                                ps[:], AF.Copy,
                                                bias=0.0, scale=WSI)
                                        else:
                                            ksb = apool.tile([P, 512], FP8,
                                                             tag="ksb")
                                            nc.scalar.activation(
                                                ksb[:], ps[:], AF.Copy,
                                                bias=0.0, scale=WSI)
                                            nc.gpsimd.dma_start(
                                                kdram[m, :, b,
                                                      ds(sh * SQ, SQ)],
                                                ksb[:])
                                        # V (token-major, 65-lane layout)
                                        tl, nb = divmod(m, D // 512)
                                        tt = sh * (SQ // P) + tl
                                        ps = psA.tile([P, 512], FP32,
                                                      tag="psa")
                                        for kt in range(0, KT, 2):
                                            nc.tensor.matmul(
                                                ps[:],
                                                xth[:, ds(kt, 2),
                                                    ts(tl, P)],
                                                wv_sb[:, ds(kt, 2),
                                                      ts(nb, 512)],
                                                start=(kt == 0),
                                                stop=(kt == KT - 2),
                                                perf_mode=DR)
                                        nc.vector.scalar_tensor_tensor(
                                            v_sb[:, b, tt,
                                                 ds(nb * 8, 8),
                                                 ds(0, DH)],
                                            ps[:], WSI,
                                            bvr_sb[:, ts(nb, 512)],
                                            ALU.mult, ALU.add)

                        # ===== Phase B: attention =====
                        # prefetch phase-C weights while attention runs
                        with tc.tile_pool(name="wo_p", bufs=1) as wopool:
                            wo_sb = wopool.tile([P, KT, D], FP8, tag="wo")
                            nc.scalar.dma_start(wo_sb[:], wo_t)
                            ident = wopool.tile([P, P], FP32, tag="ident")
                            nc.scalar.dma_start(ident[:], identd[:])
                            g1r_sb = wopool.tile([P, D], FP32, tag="g1r")
                            b1r_sb = wopool.tile([P, D], FP32, tag="b1r")
                            nc.gpsimd.dma_start(g1r_sb[:], g1_rep[:])
                            nc.gpsimd.dma_start(b1r_sb[:], b1_rep[:])

                            if "B" not in phases:
                                nc.vector.memset(oT_sb[:], 0.001)
                            with (
                                tc.tile_pool(name="kpair", bufs=3) as kpool,
                                tc.tile_pool(name="expst", bufs=3) as epool,
                                tc.tile_pool(name="battn", bufs=4) as bpool,
                                tc.tile_pool(name="psS", bufs=2,
                                             space="PSUM") as psS,
                                tc.tile_pool(name="psO", bufs=2,
                                             space="PSUM") as psO,
                                tc.tile_pool(name="psR", bufs=2,
                                             space="PSUM") as psR,
                            ):
                                hpb = [(b, hp) for b in range(B)
                                       for hp in range(HP)]
                                for b, hp in (hpb if "B" in phases else ()):
                                    if True:
                                        if hp == 0:
                                            kpair = k0_sb[:, b, :]
                                        else:
                                            kp_t = kpool.tile([P, S], FP8,
                                                              tag="kpair")
                                            nc.sync.dma_start(
                                                kp_t[:], kdram[hp, :, b, :])
                                            kpair = kp_t
                                        for h01 in range(2):
                                            po = h01 * DH
                                            h = hp * 2 + h01
                                            expst = epool.tile(
                                                [P, TT, CH], BF16,
                                                tag="expst")
                                            for g in range(TT // EG):
                                                pss = psS.tile(
                                                    [P, EG, CH], FP32,
                                                    tag="pss")
                                                for j in range(EG):
                                                    tt = g * EG + j
                                                    nc.tensor.matmul(
                                                        pss[:, j, :],
                                                        kpair[ds(po, DH),
                                                              ts(tt, P)],
                                                        qT_sb[ds(po, DH),
                                                              hp, b, :])
                                                nc.scalar.activation(
                                                    expst[:, ds(g * EG, EG),
                                                          :],
                                                    pss[:], AF.Exp)
                                            po_ps = psO.tile([VW, CH], FP32,
                                                             tag="pso")
                                            for tt in range(TT):
                                                nc.tensor.matmul(
                                                    po_ps[:],
                                                    v_sb[:, b, tt, h, :],
                                                    expst[:, tt, :],
                                                    start=(tt == 0),
                                                    stop=(tt == TT - 1))
                                            # softmax denominators sit in
                                            # row 64 (the ones lane)
                                            rsum = bpool.tile([1, CH], FP32,
                                                              tag="rsum")
                                            nc.vector.tensor_copy(
                                                rsum[:],
                                                po_ps[ds(DH, 1), :])
                                            rec = bpool.tile([1, CH], F32R,
                                                             tag="rec")
                                            with nc.allow_low_precision(
                                                    reason="f32r recip for "
                                                    "1-cyc/row broadcast"):
                                                nc.vector.reciprocal(
                                                    rec[:], rsum[:])
                                            # broadcast across partitions:
                                            # ones[1,64]^T @ rec[1,CH]
                                            recb = psR.tile([DH, CH], FP32,
                                                            tag="recb")
                                            nc.tensor.matmul(
                                                recb[:], ones_col[:],
                                                rec[:],
                                                start=True, stop=True)
                                            recb_sb = bpool.tile(
                                                [DH, CH], FP32, tag="recb_sb")
                                            nc.vector.tensor_copy(
                                                recb_sb[:], recb[:])
                                            with nc.allow_low_precision(
                                                    reason="oT stored fp8; "
                                                    "noise hidden by "
                                                    "residual LN"):
                                                nc.vector.tensor_tensor(
                                                    oT_sb[ds(po, DH),
                                                          hp, b, :],
                                                    po_ps[ds(0, DH), :],
                                                    recb_sb[:], ALU.mult)

                    # ===== Phase C: O-proj + residual + LN1 (+ x1^T) =====
                            # w2 takes over V's SBUF slot; chunks are
                            # paced through C and E to avoid a DMA spike
                            w2r_sb = bigp.tile([P, FT // 2, 2, D + 16],
                                               BF16, tag="big")
                            if "C" not in phases:
                                nc.vector.memset(x1_sb[:], 0.001)
                                nc.vector.memset(x1T_sb[:], 0.001)
                            with (
                                tc.tile_pool(name="cscr", bufs=2) as cpool,
                                tc.tile_pool(name="psC", bufs=2,
                                             space="PSUM") as psC,
                                tc.tile_pool(name="psD", bufs=2,
                                             space="PSUM") as psD,
                            ):
                                for b in range(B if "C" in phases else 0):
                                    for tq in range(B):
                                        kp = b * B + tq
                                        nc.gpsimd.dma_start(
                                            w2r_sb[:, kp, :, ds(0, D)],
                                            w2s[kp])
                                        ps = psC.tile([P, D], FP32, tag="psc")
                                        for nb in range(D // 512):
                                            for kt in range(0, KT, 2):
                                                nc.tensor.matmul(
                                                    ps[:, ts(nb, 512)],
                                                    oT_sb[:, ds(kt, 2), b,
                                                          ts(tq, P)],
                                                    wo_sb[:, ds(kt, 2),
                                                          ts(nb, 512)],
                                                    start=(kt == 0),
                                                    stop=(kt == KT - 2),
                                                    perf_mode=DR)
                                        xres_sb = cpool.tile([P, D], FP32,
                                                             tag="xres")
                                        nc.sync.dma_start(
                                            xres_sb[:], xres_t[:, b, tq, :])
                                        # fused: t1 = ps + (x + bo),
                                        # row-sum for the LN mean
                                        t1 = cpool.tile([P, D], FP32,
                                                        tag="c_t1")
                                        ssum = cpool.tile([P, 1], FP32,
                                                          tag="c_ssum")
                                        nc.vector.scalar_tensor_tensor(
                                            t1[:], ps[:], WSI, xres_sb[:],
                                            ALU.mult, ALU.add,
                                            accum_out=ssum[:])
                                        y_t = layer_norm_tail(
                                            cpool, t1, ssum, g1r_sb, b1r_sb,
                                            x1_sb[:, b, tq, :])
                                        # transposes take the PRE-affine y:
                                        # g1 is folded into W1 host-side
                                        # (bb1' = bb1 + b1@W1), so FFN1 is
                                        # exact while the affine x1 (for
                                        # the F residual) finishes in
                                        # parallel.
                                        for kd in range(KT):
                                            pt = psD.tile([P, P], FP32,
                                                          tag="psd")
                                            nc.tensor.transpose(
                                                pt[:],
                                                y_t[:, ts(kd, P)],
                                                ident[:])
                                            nc.scalar.copy(
                                                x1T_sb[:, kd, b, ts(tq, P)],
                                                pt[:])
                        apool_cm.__exit__(None, None, None)

                  # ===== Phase E: FFN1  hT = relu(x1@W1+bb1)^T =====
                  with tc.tile_pool(name="hT", bufs=1) as hpool:
                      hT_sb = hpool.tile([P, FT, TQ], BF16, tag="hT")
                      with (
                          tc.tile_pool(name="w1_p", bufs=4) as w1pool,
                          tc.tile_pool(name="psE", bufs=4,
                                       space="PSUM") as psE,
                      ):
                          if "E" not in phases:
                              nc.vector.memset(hT_sb[:], 0.001)
                          for mh in range(FT if "E" in phases else 0):
                              if mh % 2 == 0 and 4 + mh // 2 < FT // 2:
                                  kp = 4 + mh // 2
                                  nc.gpsimd.dma_start(
                                      w2r_sb[:, kp, :, ds(0, D)],
                                      w2s[kp])
                              w1t = w1pool.tile([P, KT, P], BF16, tag="w1t")
                              nc.sync.dma_start(w1t[:], w1s[mh])
                              ps = psE.tile([P, TQ], FP32, tag="pse")
                              for kt in range(KT):
                                  nc.tensor.matmul(
                                      ps[:], w1t[:, kt, :],
                                      x1T_sb[:, kt, :, :],
                                      start=(kt == 0), stop=(kt == KT - 1))
                              nc.scalar.activation(
                                  hT_sb[:, mh, :], ps[:], AF.Relu,
                                  bias=bb1_sb[:, ds(mh, 1)])

                      # ===== Phase F: FFN2 + residual + LN2 -> out =====
                      # kt-outer, two token-halves; w2 streamed per kt-pair
                      with (
                          tc.tile_pool(name="fscr", bufs=2) as fpool,
                          tc.tile_pool(name="fbias", bufs=1) as fbp,
                          tc.tile_pool(name="psF", bufs=2, space="PSUM") as psF,
                      ):
                          g2r_sb = fbp.tile([P, D], FP32, tag="g2r")
                          b2r_sb = fbp.tile([P, D], FP32, tag="b2r")
                          nc.gpsimd.dma_start(g2r_sb[:], g2_rep[:])
                          nc.gpsimd.dma_start(b2r_sb[:], b2_rep[:])
                          if "F" not in phases:
                              for b in range(B):
                                  for tq in range(B):
                                      dummy = fpool.tile([P, D], FP32,
                                                         tag="f_out")
                                      nc.vector.memset(dummy[:], 0.5)
                                      nc.sync.dma_start(out_t[:, b, tq, :],
                                                        dummy[:])
                          for c in range(B * B if "F" in phases else 0):
                              ps_f = psF.tile([P, D], FP32, tag=f"psf{c % 2}")
                              for kp in range(FT // 2):
                                  for kj in range(2):
                                      for nb in range(D // 512):
                                          nc.tensor.matmul(
                                              ps_f[:, ts(nb, 512)],
                                              hT_sb[:, 2 * kp + kj,
                                                    ts(c, P)],
                                              w2r_sb[:, kp, kj, ts(nb, 512)],
                                              start=(kp == 0 and kj == 0),
                                              stop=False)
                              # bb2 via contract-1 matmuls on the idle
                              # PE; closes the accumulation group
                              for nb in range(D // 512):
                                  nc.tensor.matmul(
                                      ps_f[:, ts(nb, 512)],
                                      ones_row[:],
                                      bb2r_sb[:, ts(nb, 512)],
                                      start=False,
                                      stop=True)
                              if True:
                                  b, tq = divmod(c, B)
                                  t1 = fpool.tile([P, D], FP32, tag="f_t1")
                                  ssum = fpool.tile([P, 1], FP32,
                                                    tag="f_ssum")
                                  nc.vector.scalar_tensor_tensor(
                                      t1[:], ps_f[:], 1.0,
                                      x1_sb[:, b, tq, :],
                                      ALU.mult, ALU.add, accum_out=ssum[:])
                                  o_sb = fpool.tile([P, D], FP32, tag="f_out")
                                  layer_norm_tail(fpool, t1, ssum,
                                                  g2r_sb, b2r_sb, o_sb,
                                                  out_halves=(out_t, b, tq))

    if split_waits:
        _split_multiwaits(nc)
    return nc


_NC_CACHE = None


def _get_bass():
    global _NC_CACHE
    if _NC_CACHE is None:
        _NC_CACHE = build_bass()
    return _NC_CACHE


def make_in_maps(x, Wq, bq, Wk, bk, Wv, bv, Wo, bo, g1, b1, W1, bb1, W2, bb2,
                 g2, b2):
    bf = ml_dtypes.bfloat16
    f8 = ml_dtypes.float8_e4m3
    x = np.asarray(x, np.float32)
    xT = np.ascontiguousarray(x.transpose(2, 1, 0))              # [D,B,S]
    W1 = np.asarray(W1, np.float32)
    W2 = np.asarray(W2, np.float32)
    g1f = np.asarray(g1, np.float32)
    b1f = np.asarray(b1, np.float32)
    bb1 = np.asarray(bb1, np.float32) + b1f @ W1
    W1 = g1f[:, None] * W1
    # w1s[mh] = W1[:, mh*128:(mh+1)*128] rearranged [(kt p), n] -> [p kt n]
    w1s = np.ascontiguousarray(
        W1.reshape(KT, P, FT, P).transpose(2, 1, 0, 3)).astype(bf)
    # w2s[kp] = W2[kp*256:(kp+1)*256, :] as [P, 2, D]
    w2s = np.ascontiguousarray(
        W2.reshape(FT // 2, 2, P, D).transpose(0, 2, 1, 3)).astype(bf)
    shared = {
        "xT8": xT.astype(f8),
        "wq8": (np.asarray(Wq, np.float32) * WS).astype(f8),
        "wk8": (np.asarray(Wk, np.float32) * WS).astype(f8),
        "wv8": (np.asarray(Wv, np.float32) * WS).astype(f8),
        "wo8": (np.asarray(Wo, np.float32) * WS).astype(f8),
        "w1s": w1s,
        "w2s": w2s,
        "ident": np.eye(P, dtype=np.float32),
        "bqs": (np.asarray(bq, np.float32) / 8.0),
        "bb1": bb1,
        "bb2r": np.asarray(bb2, np.float32).reshape(1, D),
        "bv_rep": np.tile(np.asarray(bv, np.float32), (P, 1)),
        "g1_rep": np.tile(np.asarray(g1, np.float32), (P, 1)),
        "b1_rep": np.tile(np.asarray(b1, np.float32), (P, 1)),
        "g2_rep": np.tile(np.asarray(g2, np.float32), (P, 1)),
        "b2_rep": np.tile(np.asarray(b2, np.float32), (P, 1)),
    }
    xf8 = xT.astype(f8)
    xpbo = x + np.asarray(bo, np.float32)       # fold bo into the residual
    in_maps = []
    for c in range(NC):
        sl = slice(c * CH, (c + 1) * CH)
        m = dict(shared)
        m["xTq8"] = np.ascontiguousarray(xf8[:, :, sl])
        m["xres"] = np.ascontiguousarray(
            xpbo[sl].transpose(1, 0, 2))           # [B, CH, D]
        in_maps.append(m)
    return in_maps


def assemble(results):
    out = np.empty((S, B, D), np.float32)
    for c, r in enumerate(results):
        out[c * CH:(c + 1) * CH] = r["out"].transpose(1, 0, 2)
    return out


def kernel(**inputs) -> np.ndarray:
    nc = _get_bass()
    in_maps = make_in_maps(**inputs)
    res = run_bass_kernel_spmd(nc, in_maps, core_ids=list(range(NC)))
    return assemble(res.results)



# revision 7
# speedup vs baseline: 55.2080x; 1.1114x over previous
"""Trainium2 Bass kernel for nn_EncoderBlock (dense transformer block).

Reference computation (fp32, S=2048 B=2 D=1024 H=16 dh=64 F=4096):
    q,k,v = x@Wq+bq, x@Wk+bk, x@Wv+bv          (per-head split, dh=64)
    attn  = softmax(q k^T / sqrt(dh)) v         (full S x S scores)
    o     = attn-merge @ Wo + bo
    x1    = LN(x + o; g1,b1)
    out   = LN(x1 + relu(x1@W1+bb1)@W2+bb2; g2,b2)

Sharding: sequence-parallel over 8 cores. Each core owns 256 seq positions
(x 2 batches = 512 tokens) end-to-end; K/V are computed redundantly on every
core (an on-chip collective costs more than the recompute at this size).

Precision: the Q/K/V projections and the O-projection run in fp8-e4m3
with DoubleRow perf mode (2 k-tiles per instruction, 0.5 cycles/row =>
4x bf16 throughput). QKV noise washes out in the softmax average over
2048 keys; O-proj noise rides on o, which attention-averaging leaves
small next to x in LN(x+o) (measured 1.81e-3 rel err vs 1.66e-3
all-bf16). The FFN stays bf16: fp8 there costs ~1.9e-2 rel err — ff is
comparable to x1 in magnitude and nothing averages the noise away.
Weights are pre-scaled by 64 on the host so w*64 ~ N(0,1.3) stays in
fp8's normal range; the 1/64 is folded into the fused psum-evacuation
ops. K's bias is dropped outright: it shifts all of a query's scores
equally, which softmax cancels.

Attention: scores are computed TRANSPOSED (S^T[tk,q], lhsT=K^T chunk,
rhs=Q^T chunk) so exp(S^T) feeds the PV matmul directly as the moving
operand with token-major V as the stationary one. Softmax max-subtraction
is skipped (|s| < ~10, exp cannot overflow). The softmax denominator
comes FOR FREE from a ones-lane appended to the V stationary (65-wide
stationary -> row 64 of the PV psum is the exp row-sum); the reciprocal
row is broadcast across partitions with a contract-dim-1 fp32 matmul and
multiplied out on DVE. Exps are batched 4 token-tiles per Act
instruction to amortize the ~370ns Act fixed overhead.

w1 is streamed in per-column-block tiles into FFN1; w2 takes over V's
8MB SBUF slot (disjoint lifetimes), its load paced in chunks through
phases C/E so it never spikes the DMA engines. FFN2 runs kt-outer per
token-block (4 sequential psum accumulators) so each block's LayerNorm
hides under the next block's matmuls; bb2 is added by contract-dim-1
matmuls on the otherwise-idle PE. LayerNorm evacuations are fused:
psum-scale + residual-add + row-sum in one DVE op
(scalar_tensor_tensor with accum_out), variance via E[x^2]-m^2 on Act,
and the final affine+store is halved so the out DMA starts early."""

import numpy as np
import ml_dtypes

import concourse.bass as bass
import concourse.mybir as mybir
import concourse.tile as tile
from concourse.bass import ts, ds
from concourse.bass_utils import run_bass_kernel_spmd

BF16 = mybir.dt.bfloat16
FP32 = mybir.dt.float32
FP8 = mybir.dt.float8e4
F32R = mybir.dt.float32r
AF = mybir.ActivationFunctionType
ALU = mybir.AluOpType
DR = mybir.MatmulPerfMode.DoubleRow

S, B, D, H, DH, F = 2048, 2, 1024, 16, 64, 4096
NC = 8              # cores
CH = S // NC        # seq positions per core (256)
TQ = CH * B         # tokens per core (512)
P = 128
KT = D // P         # 8 k-tiles over D
MT = D // P         # 8 m-tiles over D
FT = F // P         # 32 tiles over F
TT = S // P         # 16 token-tiles per batch
LN_EPS = 1e-5
HP = H // 2         # 8 head-pairs
VW = DH + 1         # 65: V columns + ones lane
EG = 4              # token-tiles per batched exp
WS = 64.0           # host-side fp8 weight scale
WSI = 1.0 / WS


def _split_multiwaits(nc):
    # Walrus in this container encodes at most ONE sync-wait per instruction.
    # Tile's tail drain violates that; hoist extra waits onto fresh NoOps.
    for bb in nc.m.functions[0].blocks:
        new_insts = []
        for inst in bb.instructions:
            si = inst.sync_info
            if si is not None and si.on_wait and len(si.on_wait) > 1:
                waits = list(si.on_wait)
                for j, w in enumerate(waits[:-1]):
                    new_insts.append(mybir.InstNoOp(
                        name=f"{inst.name}-wsplit{j}", engine=inst.engine,
                        ins=[], outs=[],
                        sync_info=mybir.SyncInfo(on_wait=[w], on_update=[])))
                si.on_wait = [waits[-1]]
            new_insts.append(inst)
        bb.instructions = new_insts


def build_bass(split_waits=True, phases="ABCEF"):
    nc = bass.Bass(name="encoder_block", num_devices=NC, debug=False)

    # ---- I/O ----
    xT8 = nc.dram_tensor("xT8", (D, B, S), FP8, kind="ExternalInput")
    xTq8 = nc.dram_tensor("xTq8", (D, B, CH), FP8, kind="ExternalInput")
    xres = nc.dram_tensor("xres", (B, CH, D), FP32, kind="ExternalInput")
    wq8 = nc.dram_tensor("wq8", (D, D), FP8, kind="ExternalInput")
    wk8 = nc.dram_tensor("wk8", (D, D), FP8, kind="ExternalInput")
    wv8 = nc.dram_tensor("wv8", (D, D), FP8, kind="ExternalInput")
    wo8 = nc.dram_tensor("wo8", (D, D), FP8, kind="ExternalInput")
    w1s = nc.dram_tensor("w1s", (FT, P, KT, P), BF16, kind="ExternalInput")
    w2s = nc.dram_tensor("w2s", (FT // 2, P, 2, D), BF16,
                         kind="ExternalInput")
    identd = nc.dram_tensor("ident", (P, P), FP32, kind="ExternalInput")
    bqs = nc.dram_tensor("bqs", (D,), FP32, kind="ExternalInput")  # bq/8
    bb1 = nc.dram_tensor("bb1", (F,), FP32, kind="ExternalInput")  # *WS
    bb2r = nc.dram_tensor("bb2r", (1, D), FP32, kind="ExternalInput")  # *WS
    bv_rep = nc.dram_tensor("bv_rep", (P, D), FP32, kind="ExternalInput")
    g1_rep = nc.dram_tensor("g1_rep", (P, D), FP32, kind="ExternalInput")
    b1_rep = nc.dram_tensor("b1_rep", (P, D), FP32, kind="ExternalInput")
    g2_rep = nc.dram_tensor("g2_rep", (P, D), FP32, kind="ExternalInput")
    b2_rep = nc.dram_tensor("b2_rep", (P, D), FP32, kind="ExternalInput")
    out = nc.dram_tensor("out", (B, CH, D), FP32, kind="ExternalOutput")

    xT_t = xT8.rearrange("(kt p) b s -> p kt b s", p=P)
    xTq_t = xTq8.rearrange("(kt p) b s -> p kt b s", p=P)
    xres_t = xres.rearrange("b (tq p) d -> p b tq d", p=P)
    out_t = out.rearrange("b (tq p) d -> p b tq d", p=P)
    wq_t = wq8.rearrange("(kt p) n -> p kt n", p=P)
    wk_t = wk8.rearrange("(kt p) n -> p kt n", p=P)
    wv_t = wv8.rearrange("(kt p) n -> p kt n", p=P)
    wo_t = wo8.rearrange("(kt p) n -> p kt n", p=P)
    bqs_t = bqs.rearrange("(m p) -> p m", p=P)
    bb1_t = bb1.rearrange("(m p) -> p m", p=P)

    eps_sb = None

    def layer_norm_tail(pool, t1, ssum, g_sb, bt_sb, dst,
                        out_halves=None):
        """dst = LN(t1)*g + bt given t1 [P,D] fp32 and its row-sum ssum.

        Variance via E[x^2] - m^2: one Act Square pass over a scratch,
        tiny per-partition fixups, then a single fused (t1-m)*rstd DVE op.
        """
        sq = pool.tile([P, D], FP32, tag="ln_sq")
        ss = pool.tile([P, 1], FP32, tag="ln_ss")
        nc.scalar.activation(sq[:], t1[:], AF.Square, accum_out=ss[:])
        negmean = pool.tile([P, 1], FP32, tag="ln_negmean")
        nc.scalar.mul(negmean[:], ssum[:], -1.0 / D)
        m2 = pool.tile([P, 1], FP32, tag="ln_m2")
        nc.scalar.activation(m2[:], negmean[:], AF.Square)
        eb = pool.tile([P, 1], FP32, tag="ln_eb")
        nc.vector.tensor_scalar(eb[:], m2[:], -1.0, LN_EPS, ALU.mult,
                                ALU.add)
        st = pool.tile([P, 1], FP32, tag="ln_st")
        nc.scalar.activation(st[:], ss[:], AF.Sqrt, bias=eb[:],
                             scale=1.0 / D)
        rstd = pool.tile([P, 1], FP32, tag="ln_rstd")
        nc.vector.reciprocal(rstd[:], st[:])
        y = pool.tile([P, D], FP32, tag="ln_y")
        nc.vector.tensor_scalar(y[:], t1[:], negmean[:], rstd[:],
                                ALU.add, ALU.mult)
        if out_halves is None:
            yg = pool.tile([P, D], FP32, tag="ln_yg")
            nc.vector.tensor_tensor(yg[:], y[:], g_sb[:], ALU.mult)
            nc.vector.tensor_tensor(dst[:], yg[:], bt_sb[:], ALU.add)
        else:
            # halved final affine, each half DMA'd out immediately
            out_t, b, tq = out_halves
            for nbh in range(2):
                h = ds(nbh * (D // 2), D // 2)
                nc.vector.tensor_tensor(y[:, h], y[:, h], g_sb[:, h],
                                        ALU.mult)
                nc.vector.tensor_tensor(dst[:, h], y[:, h], bt_sb[:, h],
                                        ALU.add)
                nc.sync.dma_start(out_t[:, b, tq, h], dst[:, h])
        return y

    with tile.TileContext(nc) as tc:
        with (
            tc.tile_pool(name="persist", bufs=1) as pp,
            tc.tile_pool(name="dram", bufs=1, space="DRAM") as dpool,
        ):
            # alive for the whole kernel
            bqs_sb = pp.tile([P, MT], FP32, tag="bqs")
            bb1_sb = pp.tile([P, FT], FP32, tag="bb1")
            bb2r_sb = pp.tile([1, D], FP32, tag="bb2r")
            ones_row = pp.tile([1, P], FP32, tag="ones_row")

            kdram = dpool.tile([HP, P, B, S], FP8)               # K^T spill

            eps_sb = pp.tile([P, 1], FP32, tag="eps")
            nc.vector.memset(eps_sb[:], LN_EPS)
            nc.vector.memset(ones_row[:], 1.0)
            nc.gpsimd.dma_start(bqs_sb[:], bqs_t)
            nc.gpsimd.dma_start(bb1_sb[:], bb1_t)
            nc.gpsimd.dma_start(bb2r_sb[:], bb2r[:])

            with tc.tile_pool(name="x1p", bufs=1) as x1p:
                # alive A..F
                x1_sb = x1p.tile([P, B, B, D], FP32, tag="x1")
                x1T_sb = x1p.tile([P, KT, B, CH], BF16, tag="x1T")

                with tc.tile_pool(name="bigp", bufs=1) as bigp:
                  # one big slot reused across phases: V (A..B), then w2 (C..F)
                  v_sb = bigp.tile([P, B, TT, H, VW], BF16, tag="big")
                  with tc.tile_pool(name="otx", bufs=1) as otx:
                    # alive A..C (1 MB)
                    oT_sb = otx.tile([P, MT, B, CH], FP8, tag="oT")

                    with tc.tile_pool(name="vq", bufs=1) as vq:
                        # zero-padded Q^T: per head-pair, head0 queries in
                        # cols 0:CH (partitions 64:128 zero), head1 queries
                        # in cols CH:2CH (partitions 0:64 zero). One scores
                        # matmul then serves BOTH heads with free=512: the
                        # complementary kpair rows hit exact zeros.
                        qT_sb = vq.tile([P, HP, B, 2 * CH], BF16, tag="qT")
                        k0_sb = vq.tile([P, B, S], FP8, tag="k0")
                        bvr_sb = vq.tile([P, D], FP32, tag="bvr")
                        ones_col = vq.tile([1, DH], F32R, tag="ones_col")
                        ones_f32 = vq.tile([1, DH], FP32, tag="ones_f32")
                        nc.gpsimd.dma_start(bvr_sb[:], bv_rep[:])
                        nc.vector.memset(ones_f32[:], 1.0)
                        with nc.allow_low_precision(
                                reason="f32r ones for 1-cyc/row broadcast"):
                            nc.vector.tensor_copy(ones_col[:], ones_f32[:])
                        # ones lane for the fused softmax row-sum
                        # (on the otherwise-idle gpsimd engine)
                        nc.gpsimd.memset(
                            v_sb[:, :, :, :, ds(DH, 1)], 1.0)

                        # ===== Phase A: projections (K^T, V, Q^T) =====
                        # aout outlives A: its ksb tiles' last readers are
                        # kdram-write DMAs that drain late; keeping the
                        # pool open stops B's pools from WAR-waiting on
                        # that space.
                        apool_cm = tc.tile_pool(name="aout", bufs=6)
                        apool = apool_cm.__enter__()
                        with (
                            tc.tile_pool(name="wqp", bufs=1) as wqpool,
                            tc.tile_pool(name="wqkv", bufs=2) as wpool,
                            tc.tile_pool(name="xt", bufs=3) as xpool,
                            tc.tile_pool(name="psA", bufs=5,
                                         space="PSUM") as psA,
                        ):
                            # initial loads fan out over the DMA queues
                            wq_sb = wqpool.tile([P, KT, D], FP8, tag="wq")
                            nc.gpsimd.dma_start(wq_sb[:], wq_t)
                            wk_sb = wpool.tile([P, KT, D], FP8, tag="w")
                            nc.sync.dma_start(wk_sb[:], wk_t)
                            wv_sb = wpool.tile([P, KT, D], FP8, tag="w")
                            nc.scalar.dma_start(wv_sb[:], wv_t)
                            xtq_sb = xpool.tile([P, KT, B, CH], FP8,
                                                tag="xtq")
                            nc.gpsimd.dma_start(xtq_sb[:], xTq_t)

                            # Q^T first (only needs wq+xtq): its DVE
                            # evacs land early so phase B's first scores
                            # don't wait on A's whole DVE queue, and the
                            # matmuls fill the initial weight-DMA window.
                            nc.gpsimd.memset(
                                qT_sb[ds(0, DH), :, :, ds(CH, CH)], 0.0)
                            nc.gpsimd.memset(
                                qT_sb[ds(DH, DH), :, :, ds(0, CH)], 0.0)
                            for m in range(MT):
                                # both batches per matmul: free=512 halves
                                # the ldweights count
                                ps = psA.tile([P, B, CH], FP32, tag="psq",
                                              bufs=2)
                                for kt in range(0, KT, 2):
                                    nc.tensor.matmul(
                                        ps[:],
                                        wq_sb[:, ds(kt, 2), ts(m, P)],
                                        xtq_sb[:, ds(kt, 2), :, :],
                                        start=(kt == 0),
                                        stop=(kt == KT - 2),
                                        perf_mode=DR)
                                for b in range(B):
                                    for h01 in range(2):
                                        po = h01 * DH
                                        nc.vector.tensor_scalar(
                                            qT_sb[ds(po, DH), m, b,
                                                  ds(h01 * CH, CH)],
                                            ps[ds(po, DH), b, :],
                                            0.125 * WSI,
                                            bqs_sb[ds(po, DH), ds(m, 1)],
                                            ALU.mult, ALU.add)

                            SQ = S // 2  # 1024-token stream chunks
                            for b in range(B):
                                for sh in range(2):
                                    xth = xpool.tile([P, KT, SQ], FP8,
                                                     tag="xth")
                                    (nc.scalar if (b + sh) == 0 else
                                     nc.sync).dma_start(
                                        xth[:],
                                        xT_t[:, :, b, ds(sh * SQ, SQ)])
                                    # K^T and V interleaved so the Act
                                    # (K-evac) and DVE (V-evac) engines
                                    # alternate and psum banks recycle
                                    # without bursty evac lag.
                                    # NOTE: bk is dropped on purpose:
                                    # K's bias adds q.bk to every score
                                    # of a query, which softmax cancels.
                                    for m in range(MT):
                                        ps = psA.tile([P, SQ], FP32,
                                                      tag="psa", bufs=3)
                                        for kt in range(0, KT, 2):
                                            nc.tensor.matmul(
                                                ps[:],
                                                wk_sb[:, ds(kt, 2),
                                                      ts(m, P)],
                                                xth[:, ds(kt, 2), :],
                                                start=(kt == 0),
                                                stop=(kt == KT - 2),
                                                perf_mode=DR)
                                        if m == 0:
                                            nc.scalar.activation(
                                                k0_sb[:, b, ds(sh * SQ, SQ)],
                                                ps[:], AF.Copy,
                                                bias=0.0, scale=WSI)
                                        else:
                                            ksb = apool.tile([P, SQ], FP8,
                                                             tag="ksb")
                                            nc.scalar.activation(
                                                ksb[:], ps[:], AF.Copy,
                                                bias=0.0, scale=WSI)
                                            nc.gpsimd.dma_start(
                                                kdram[m, :, b,
                                                      ds(sh * SQ, SQ)],
                                                ksb[:])
                                        # V (token-major, 65-lane layout):
                                        # full-D moving (free=1024), one
                                        # psum per token-tile
                                        tt = sh * (SQ // P) + m
                                        ps = psA.tile([P, SQ], FP32,
                                                      tag="psa", bufs=3)
                                        for kt in range(0, KT, 2):
                                            nc.tensor.matmul(
                                                ps[:],
                                                xth[:, ds(kt, 2),
                                                    ts(m, P)],
                                                wv_sb[:, ds(kt, 2), :],
                                                start=(kt == 0),
                                                stop=(kt == KT - 2),
                                                perf_mode=DR)
                                        nc.vector.scalar_tensor_tensor(
                                            v_sb[:, b, tt, :,
                                                 ds(0, DH)],
                                            ps[:], WSI,
                                            bvr_sb[:],
                                            ALU.mult, ALU.add)

                        # ===== Phase B: attention =====
                        # prefetch phase-C weights while attention runs
                        with tc.tile_pool(name="wo_p", bufs=1) as wopool:
                            wo_sb = wopool.tile([P, KT, D], FP8, tag="wo")
                            nc.scalar.dma_start(wo_sb[:], wo_t)
                            ident = wopool.tile([P, P], FP32, tag="ident")
                            nc.scalar.dma_start(ident[:], identd[:])
                            g1r_sb = wopool.tile([P, D], FP32, tag="g1r")
                            b1r_sb = wopool.tile([P, D], FP32, tag="b1r")
                            nc.gpsimd.dma_start(g1r_sb[:], g1_rep[:])
                            nc.gpsimd.dma_start(b1r_sb[:], b1_rep[:])

                            if "B" not in phases:
                                nc.vector.memset(oT_sb[:], 0.001)
                            with (
                                tc.tile_pool(name="kpair", bufs=3) as kpool,
                                tc.tile_pool(name="expst", bufs=2) as epool,
                                tc.tile_pool(name="battn", bufs=4) as bpool,
                                tc.tile_pool(name="psS", bufs=2,
                                             space="PSUM") as psS,
                                tc.tile_pool(name="psO", bufs=2,
                                             space="PSUM") as psO,
                                tc.tile_pool(name="psR", bufs=2,
                                             space="PSUM") as psR,
                            ):
                                hpb = [(b, hp) for b in range(B)
                                       for hp in range(HP)]
                                for b, hp in (hpb if "B" in phases else ()):
                                    if True:
                                        if hp == 0:
                                            kpair = k0_sb[:, b, :]
                                        else:
                                            kp_t = kpool.tile([P, S], FP8,
                                                              tag="kpair")
                                            nc.sync.dma_start(
                                                kp_t[:], kdram[hp, :, b, :])
                                            kpair = kp_t
                                        # scores for BOTH heads at once:
                                        # kpair full 128 partitions vs the
                                        # zero-padded qT -> S^T_h0 in cols
                                        # 0:CH, S^T_h1 in cols CH:2CH.
                                        expst = epool.tile(
                                            [P, TT, 2 * CH], BF16,
                                            tag="expst")
                                        EG2 = 2
                                        for g in range(TT // EG2):
                                            pss = psS.tile(
                                                [P, EG2, 2 * CH], FP32,
                                                tag="pss")
                                            for j in range(EG2):
                                                tt = g * EG2 + j
                                                nc.tensor.matmul(
                                                    pss[:, j, :],
                                                    kpair[:, ts(tt, P)],
                                                    qT_sb[:, hp, b, :])
                                            nc.scalar.activation(
                                                expst[:, ds(g * EG2, EG2),
                                                      :],
                                                pss[:], AF.Exp)
                                        for h01 in range(2):
                                            po = h01 * DH
                                            h = hp * 2 + h01
                                            po_ps = psO.tile([VW, CH], FP32,
                                                             tag="pso")
                                            for tt in range(TT):
                                                nc.tensor.matmul(
                                                    po_ps[:],
                                                    v_sb[:, b, tt, h, :],
                                                    expst[:, tt,
                                                          ds(h01 * CH, CH)],
                                                    start=(tt == 0),
                                                    stop=(tt == TT - 1))
                                            # softmax denominators sit in
                                            # row 64 (the ones lane)
                                            rsum = bpool.tile([1, CH], FP32,
                                                              tag="rsum")
                                            nc.vector.tensor_copy(
                                                rsum[:],
                                                po_ps[ds(DH, 1), :])
                                            rec = bpool.tile([1, CH], F32R,
                                                             tag="rec")
                                            with nc.allow_low_precision(
                                                    reason="f32r recip for "
                                                    "1-cyc/row broadcast"):
                                                nc.vector.reciprocal(
                                                    rec[:], rsum[:])
                                            # broadcast across partitions:
                                            # ones[1,64]^T @ rec[1,CH]
                                            recb = psR.tile([DH, CH], FP32,
                                                            tag="recb")
                                            nc.tensor.matmul(
                                                recb[:], ones_col[:],
                                                rec[:],
                                                start=True, stop=True)
                                            recb_sb = bpool.tile(
                                                [DH, CH], FP32, tag="recb_sb")
                                            nc.vector.tensor_copy(
                                                recb_sb[:], recb[:])
                                            with nc.allow_low_precision(
                                                    reason="oT stored fp8; "
                                                    "noise hidden by "
                                                    "residual LN"):
                                                nc.vector.tensor_tensor(
                                                    oT_sb[ds(po, DH),
                                                          hp, b, :],
                                                    po_ps[ds(0, DH), :],
                                                    recb_sb[:], ALU.mult)

                    # ===== Phase C: O-proj + residual + LN1 (+ x1^T) =====
                            # w2 takes over V's SBUF slot; chunks are
                            # paced through C and E to avoid a DMA spike
                            w2r_sb = bigp.tile([P, FT // 2, 2, D + 16],
                                               BF16, tag="big")
                            if "C" not in phases:
                                nc.vector.memset(x1_sb[:], 0.001)
                                nc.vector.memset(x1T_sb[:], 0.001)
                            with (
                                tc.tile_pool(name="cscr", bufs=2) as cpool,
                                tc.tile_pool(name="psC", bufs=2,
                                             space="PSUM") as psC,
                                tc.tile_pool(name="psD", bufs=2,
                                             space="PSUM") as psD,
                            ):
                                for b in range(B if "C" in phases else 0):
                                    for tq in range(B):
                                        kp = b * B + tq
                                        nc.gpsimd.dma_start(
                                            w2r_sb[:, kp, :, ds(0, D)],
                                            w2s[kp])
                                        ps = psC.tile([P, D], FP32, tag="psc")
                                        for nb in range(D // 512):
                                            for kt in range(0, KT, 2):
                                                nc.tensor.matmul(
                                                    ps[:, ts(nb, 512)],
                                                    oT_sb[:, ds(kt, 2), b,
                                                          ts(tq, P)],
                                                    wo_sb[:, ds(kt, 2),
                                                          ts(nb, 512)],
                                                    start=(kt == 0),
                                                    stop=(kt == KT - 2),
                                                    perf_mode=DR)
                                        xres_sb = cpool.tile([P, D], FP32,
                                                             tag="xres")
                                        nc.sync.dma_start(
                                            xres_sb[:], xres_t[:, b, tq, :])
                                        # fused: t1 = ps + (x + bo),
                                        # row-sum for the LN mean
                                        t1 = cpool.tile([P, D], FP32,
                                                        tag="c_t1")
                                        ssum = cpool.tile([P, 1], FP32,
                                                          tag="c_ssum")
                                        nc.vector.scalar_tensor_tensor(
                                            t1[:], ps[:], WSI, xres_sb[:],
                                            ALU.mult, ALU.add,
                                            accum_out=ssum[:])
                                        y_t = layer_norm_tail(
                                            cpool, t1, ssum, g1r_sb, b1r_sb,
                                            x1_sb[:, b, tq, :])
                                        # transposes take the PRE-affine y:
                                        # g1 is folded into W1 host-side
                                        # (bb1' = bb1 + b1@W1), so FFN1 is
                                        # exact while the affine x1 (for
                                        # the F residual) finishes in
                                        # parallel.
                                        for kd in range(KT):
                                            pt = psD.tile([P, P], FP32,
                                                          tag="psd")
                                            nc.tensor.transpose(
                                                pt[:],
                                                y_t[:, ts(kd, P)],
                                                ident[:])
                                            nc.scalar.copy(
                                                x1T_sb[:, kd, b, ts(tq, P)],
                                                pt[:])
                        apool_cm.__exit__(None, None, None)

                  # ===== Phase E: FFN1  hT = relu(x1@W1+bb1)^T =====
                  with tc.tile_pool(name="hT", bufs=1) as hpool:
                      hT_sb = hpool.tile([P, FT, TQ], BF16, tag="hT")
                      with (
                          tc.tile_pool(name="w1_p", bufs=4) as w1pool,
                          tc.tile_pool(name="psE", bufs=4,
                                       space="PSUM") as psE,
                      ):
                          if "E" not in phases:
                              nc.vector.memset(hT_sb[:], 0.001)
                          for mh in range(FT if "E" in phases else 0):
                              if mh % 2 == 0 and 4 + mh // 2 < FT // 2:
                                  kp = 4 + mh // 2
                                  nc.gpsimd.dma_start(
                                      w2r_sb[:, kp, :, ds(0, D)],
                                      w2s[kp])
                              w1t = w1pool.tile([P, KT, P], BF16, tag="w1t")
                              nc.sync.dma_start(w1t[:], w1s[mh])
                              ps = psE.tile([P, TQ], FP32, tag="pse")
                              for kt in range(KT):
                                  nc.tensor.matmul(
                                      ps[:], w1t[:, kt, :],
                                      x1T_sb[:, kt, :, :],
                                      start=(kt == 0), stop=(kt == KT - 1))
                              nc.scalar.activation(
                                  hT_sb[:, mh, :], ps[:], AF.Relu,
                                  bias=bb1_sb[:, ds(mh, 1)])

                      # ===== Phase F: FFN2 + residual + LN2 -> out =====
                      # kt-outer, two token-halves; w2 streamed per kt-pair
                      with (
                          tc.tile_pool(name="fscr", bufs=2) as fpool,
                          tc.tile_pool(name="fbias", bufs=1) as fbp,
                          tc.tile_pool(name="psF", bufs=2, space="PSUM") as psF,
                      ):
                          g2r_sb = fbp.tile([P, D], FP32, tag="g2r")
                          b2r_sb = fbp.tile([P, D], FP32, tag="b2r")
                          nc.gpsimd.dma_start(g2r_sb[:], g2_rep[:])
                          nc.gpsimd.dma_start(b2r_sb[:], b2_rep[:])
                          if "F" not in phases:
                              for b in range(B):
                                  for tq in range(B):
                                      dummy = fpool.tile([P, D], FP32,
                                                         tag="f_out")
                                      nc.vector.memset(dummy[:], 0.5)
                                      nc.sync.dma_start(out_t[:, b, tq, :],
                                                        dummy[:])
                          for c in range(B * B if "F" in phases else 0):
                              ps_f = psF.tile([P, D], FP32, tag=f"psf{c % 2}")
                              for kp in range(FT // 2):
                                  for kj in range(2):
                                      for nb in range(D // 512):
                                          nc.tensor.matmul(
                                              ps_f[:, ts(nb, 512)],
                                              hT_sb[:, 2 * kp + kj,
                                                    ts(c, P)],
                                              w2r_sb[:, kp, kj, ts(nb, 512)],
                                              start=(kp == 0 and kj == 0),
                                              stop=False)
                              # bb2 via contract-1 matmuls on the idle
                              # PE; closes the accumulation group
                              for nb in range(D // 512):
                                  nc.tensor.matmul(
                                      ps_f[:, ts(nb, 512)],
                                      ones_row[:],
                                      bb2r_sb[:, ts(nb, 512)],
                                      start=False,
                                      stop=True)
                              if True:
                                  b, tq = divmod(c, B)
                                  t1 = fpool.tile([P, D], FP32, tag="f_t1")
                                  ssum = fpool.tile([P, 1], FP32,
                                                    tag="f_ssum")
                                  nc.vector.scalar_tensor_tensor(
                                      t1[:], ps_f[:], 1.0,
                                      x1_sb[:, b, tq, :],
                                      ALU.mult, ALU.add, accum_out=ssum[:])
                                  o_sb = fpool.tile([P, D], FP32, tag="f_out")
                                  layer_norm_tail(fpool, t1, ssum,
                                                  g2r_sb, b2r_sb, o_sb,
                                                  out_halves=(out_t, b, tq))

    if split_waits:
        _split_multiwaits(nc)
    return nc


_NC_CACHE = None


def _get_bass():
    global _NC_CACHE
    if _NC_CACHE is None:
        _NC_CACHE = build_bass()
    return _NC_CACHE


def make_in_maps(x, Wq, bq, Wk, bk, Wv, bv, Wo, bo, g1, b1, W1, bb1, W2, bb2,
                 g2, b2):
    bf = ml_dtypes.bfloat16
    f8 = ml_dtypes.float8_e4m3
    x = np.asarray(x, np.float32)
    xT = np.ascontiguousarray(x.transpose(2, 1, 0))              # [D,B,S]
    W1 = np.asarray(W1, np.float32)
    W2 = np.asarray(W2, np.float32)
    g1f = np.asarray(g1, np.float32)
    b1f = np.asarray(b1, np.float32)
    bb1 = np.asarray(bb1, np.float32) + b1f @ W1
    W1 = g1f[:, None] * W1
    # w1s[mh] = W1[:, mh*128:(mh+1)*128] rearranged [(kt p), n] -> [p kt n]
    w1s = np.ascontiguousarray(
        W1.reshape(KT, P, FT, P).transpose(2, 1, 0, 3)).astype(bf)
    # w2s[kp] = W2[kp*256:(kp+1)*256, :] as [P, 2, D]
    w2s = np.ascontiguousarray(
        W2.reshape(FT // 2, 2, P, D).transpose(0, 2, 1, 3)).astype(bf)
    shared = {
        "xT8": xT.astype(f8),
        "wq8": (np.asarray(Wq, np.float32) * WS).astype(f8),
        "wk8": (np.asarray(Wk, np.float32) * WS).astype(f8),
        "wv8": (np.asarray(Wv, np.float32) * WS).astype(f8),
        "wo8": (np.asarray(Wo, np.float32) * WS).astype(f8),
        "w1s": w1s,
        "w2s": w2s,
        "ident": np.eye(P, dtype=np.float32),
        "bqs": (np.asarray(bq, np.float32) / 8.0),
        "bb1": bb1,
        "bb2r": np.asarray(bb2, np.float32).reshape(1, D),
        "bv_rep": np.tile(np.asarray(bv, np.float32), (P, 1)),
        "g1_rep": np.tile(np.asarray(g1, np.float32), (P, 1)),
        "b1_rep": np.tile(np.asarray(b1, np.float32), (P, 1)),
        "g2_rep": np.tile(np.asarray(g2, np.float32), (P, 1)),
        "b2_rep": np.tile(np.asarray(b2, np.float32), (P, 1)),
    }
    xf8 = xT.astype(f8)
    xpbo = x + np.asarray(bo, np.float32)       # fold bo into the residual
    in_maps = []
    for c in range(NC):
        sl = slice(c * CH, (c + 1) * CH)
        m = dict(shared)
        m["xTq8"] = np.ascontiguousarray(xf8[:, :, sl])
        m["xres"] = np.ascontiguousarray(
            xpbo[sl].transpose(1, 0, 2))           # [B, CH, D]
        in_maps.append(m)
    return in_maps


def assemble(results):
    out = np.empty((S, B, D), np.float32)
    for c, r in enumerate(results):
        out[c * CH:(c + 1) * CH] = r["out"].transpose(1, 0, 2)
    return out


def kernel(**inputs) -> np.ndarray:
    nc = _get_bass()
    in_maps = make_in_maps(**inputs)
    res = run_bass_kernel_spmd(nc, in_maps, core_ids=list(range(NC)))
    return assemble(res.results)



# revision 11
# speedup vs baseline: 57.3499x; 1.0388x over previous
"""Trainium2 Bass kernel for nn_EncoderBlock (dense transformer block).

Reference computation (fp32, S=2048 B=2 D=1024 H=16 dh=64 F=4096):
    q,k,v = x@Wq+bq, x@Wk+bk, x@Wv+bv          (per-head split, dh=64)
    attn  = softmax(q k^T / sqrt(dh)) v         (full S x S scores)
    o     = attn-merge @ Wo + bo
    x1    = LN(x + o; g1,b1)
    out   = LN(x1 + relu(x1@W1+bb1)@W2+bb2; g2,b2)

Sharding: sequence-parallel over 8 cores. Each core owns 256 seq positions
(x 2 batches = 512 tokens) end-to-end; K/V are computed redundantly on every
core (an on-chip collective costs more than the recompute at this size).

Precision: the Q/K/V projections and the O-projection run in fp8-e4m3
with DoubleRow perf mode (2 k-tiles per instruction, 0.5 cycles/row =>
4x bf16 throughput). QKV noise washes out in the softmax average over
2048 keys; O-proj noise rides on o, which attention-averaging leaves
small next to x in LN(x+o) (measured 1.81e-3 rel err vs 1.66e-3
all-bf16). The FFN stays bf16: fp8 there costs ~1.9e-2 rel err — ff is
comparable to x1 in magnitude and nothing averages the noise away.
Weights are pre-scaled by 64 on the host so w*64 ~ N(0,1.3) stays in
fp8's normal range; the 1/64 is folded into the fused psum-evacuation
ops. K's bias is dropped outright: it shifts all of a query's scores
equally, which softmax cancels.

Attention: scores are computed TRANSPOSED (S^T[tk,q], lhsT=K^T chunk,
rhs=Q^T chunk) so exp(S^T) feeds the PV matmul directly as the moving
operand with token-major V as the stationary one. Softmax max-subtraction
is skipped (|s| < ~10, exp cannot overflow). The softmax denominator
comes FOR FREE from a ones-lane appended to the V stationary (65-wide
stationary -> row 64 of the PV psum is the exp row-sum); the reciprocal
row is broadcast across partitions with a contract-dim-1 fp32 matmul and
multiplied out on DVE. Exps are batched 4 token-tiles per Act
instruction to amortize the ~370ns Act fixed overhead.

w1 is streamed in per-column-block tiles into FFN1; w2 takes over V's
8MB SBUF slot (disjoint lifetimes), its load paced in chunks through
phases C/E so it never spikes the DMA engines. FFN2 runs kt-outer per
token-block (4 sequential psum accumulators) so each block's LayerNorm
hides under the next block's matmuls; bb2 is added by contract-dim-1
matmuls on the otherwise-idle PE. LayerNorm evacuations are fused:
psum-scale + residual-add + row-sum in one DVE op
(scalar_tensor_tensor with accum_out), variance via E[x^2]-m^2 on Act,
and the final affine+store is halved so the out DMA starts early."""

import numpy as np
import ml_dtypes

import concourse.bass as bass
import concourse.mybir as mybir
import concourse.tile as tile
from concourse.bass import ts, ds
from concourse.bass_utils import run_bass_kernel_spmd

BF16 = mybir.dt.bfloat16
FP32 = mybir.dt.float32
FP8 = mybir.dt.float8e4
F32R = mybir.dt.float32r
AF = mybir.ActivationFunctionType
ALU = mybir.AluOpType
DR = mybir.MatmulPerfMode.DoubleRow

S, B, D, H, DH, F = 2048, 2, 1024, 16, 64, 4096
NC = 8              # cores
CH = S // NC        # seq positions per core (256)
TQ = CH * B         # tokens per core (512)
P = 128
KT = D // P         # 8 k-tiles over D
MT = D // P         # 8 m-tiles over D
FT = F // P         # 32 tiles over F
TT = S // P         # 16 token-tiles per batch
LN_EPS = 1e-5
HP = H // 2         # 8 head-pairs
VW = DH + 1         # 65: V columns + ones lane
EG = 4              # token-tiles per batched exp
WS = 64.0           # host-side fp8 weight scale
WSI = 1.0 / WS


def _split_multiwaits(nc):
    # Walrus in this container encodes at most ONE sync-wait per instruction.
    # Tile's tail drain violates that; hoist extra waits onto fresh NoOps.
    for bb in nc.m.functions[0].blocks:
        new_insts = []
        for inst in bb.instructions:
            si = inst.sync_info
            if si is not None and si.on_wait and len(si.on_wait) > 1:
                waits = list(si.on_wait)
                for j, w in enumerate(waits[:-1]):
                    new_insts.append(mybir.InstNoOp(
                        name=f"{inst.name}-wsplit{j}", engine=inst.engine,
                        ins=[], outs=[],
                        sync_info=mybir.SyncInfo(on_wait=[w], on_update=[])))
                si.on_wait = [waits[-1]]
            new_insts.append(inst)
        bb.instructions = new_insts


def build_bass(split_waits=True, phases="ABCEF"):
    nc = bass.Bass(name="encoder_block", num_devices=NC, debug=False)

    # ---- I/O ----
    xT8 = nc.dram_tensor("xT8", (D, B, S), FP8, kind="ExternalInput")
    xTq8 = nc.dram_tensor("xTq8", (D, B, CH), FP8, kind="ExternalInput")
    xres = nc.dram_tensor("xres", (B, CH, D), FP32, kind="ExternalInput")
    wq8 = nc.dram_tensor("wq8", (D, D), FP8, kind="ExternalInput")
    wk8 = nc.dram_tensor("wk8", (D, D), FP8, kind="ExternalInput")
    wv8 = nc.dram_tensor("wv8", (D, D), FP8, kind="ExternalInput")
    wo8 = nc.dram_tensor("wo8", (D, D), FP8, kind="ExternalInput")
    w1s = nc.dram_tensor("w1s", (FT, P, KT, P), BF16, kind="ExternalInput")
    w2s = nc.dram_tensor("w2s", (FT // 2, P, 2, D), BF16,
                         kind="ExternalInput")
    identd = nc.dram_tensor("ident", (P, P), FP32, kind="ExternalInput")
    bqs = nc.dram_tensor("bqs", (D,), FP32, kind="ExternalInput")  # bq/8
    bb1 = nc.dram_tensor("bb1", (F,), FP32, kind="ExternalInput")  # *WS
    bb2r = nc.dram_tensor("bb2r", (1, D), FP32, kind="ExternalInput")  # *WS
    bv_rep = nc.dram_tensor("bv_rep", (P, D), FP32, kind="ExternalInput")
    g1_rep = nc.dram_tensor("g1_rep", (P, D), FP32, kind="ExternalInput")
    b1_rep = nc.dram_tensor("b1_rep", (P, D), FP32, kind="ExternalInput")
    g2_rep = nc.dram_tensor("g2_rep", (P, D), FP32, kind="ExternalInput")
    b2_rep = nc.dram_tensor("b2_rep", (P, D), FP32, kind="ExternalInput")
    out = nc.dram_tensor("out", (B, CH, D), FP32, kind="ExternalOutput")

    xT_t = xT8.rearrange("(kt p) b s -> p kt b s", p=P)
    xTq_t = xTq8.rearrange("(kt p) b s -> p kt b s", p=P)
    xres_t = xres.rearrange("b (tq p) d -> p b tq d", p=P)
    out_t = out.rearrange("b (tq p) d -> p b tq d", p=P)
    wq_t = wq8.rearrange("(kt p) n -> p kt n", p=P)
    wk_t = wk8.rearrange("(kt p) n -> p kt n", p=P)
    wv_t = wv8.rearrange("(kt p) n -> p kt n", p=P)
    wo_t = wo8.rearrange("(kt p) n -> p kt n", p=P)
    bqs_t = bqs.rearrange("(m p) -> p m", p=P)
    bb1_t = bb1.rearrange("(m p) -> p m", p=P)

    eps_sb = None

    def layer_norm_tail(pool, t1, ssum, g_sb, bt_sb, dst,
                        out_halves=None):
        """dst = LN(t1)*g + bt given t1 [P,D] fp32 and its row-sum ssum.

        Variance via E[x^2] - m^2: one Act Square pass over a scratch,
        tiny per-partition fixups, then a single fused (t1-m)*rstd DVE op.
        """
        sq = pool.tile([P, D], FP32, tag="ln_sq")
        ss = pool.tile([P, 1], FP32, tag="ln_ss")
        nc.scalar.activation(sq[:], t1[:], AF.Square, accum_out=ss[:])
        negmean = pool.tile([P, 1], FP32, tag="ln_negmean")
        nc.scalar.mul(negmean[:], ssum[:], -1.0 / D)
        m2 = pool.tile([P, 1], FP32, tag="ln_m2")
        nc.scalar.activation(m2[:], negmean[:], AF.Square)
        eb = pool.tile([P, 1], FP32, tag="ln_eb")
        nc.vector.tensor_scalar(eb[:], m2[:], -1.0, LN_EPS, ALU.mult,
                                ALU.add)
        st = pool.tile([P, 1], FP32, tag="ln_st")
        nc.scalar.activation(st[:], ss[:], AF.Sqrt, bias=eb[:],
                             scale=1.0 / D)
        rstd = pool.tile([P, 1], FP32, tag="ln_rstd")
        nc.vector.reciprocal(rstd[:], st[:])
        y = pool.tile([P, D], FP32, tag="ln_y")
        nc.vector.tensor_scalar(y[:], t1[:], negmean[:], rstd[:],
                                ALU.add, ALU.mult)
        if out_halves is None:
            yg = pool.tile([P, D], FP32, tag="ln_yg")
            nc.vector.tensor_tensor(yg[:], y[:], g_sb[:], ALU.mult)
            nc.vector.tensor_tensor(dst[:], yg[:], bt_sb[:], ALU.add)
        else:
            # halved final affine, each half DMA'd out immediately
            out_t, b, tq = out_halves
            for nbh in range(2):
                h = ds(nbh * (D // 2), D // 2)
                nc.vector.tensor_tensor(y[:, h], y[:, h], g_sb[:, h],
                                        ALU.mult)
                nc.vector.tensor_tensor(dst[:, h], y[:, h], bt_sb[:, h],
                                        ALU.add)
                nc.sync.dma_start(out_t[:, b, tq, h], dst[:, h])
        return y

    with tile.TileContext(nc) as tc:
        with (
            tc.tile_pool(name="persist", bufs=1) as pp,
            tc.tile_pool(name="dram", bufs=1, space="DRAM") as dpool,
        ):
            # alive for the whole kernel
            bqs_sb = pp.tile([P, MT], FP32, tag="bqs")
            bb1_sb = pp.tile([P, FT], FP32, tag="bb1")
            bb2r_sb = pp.tile([1, D], FP32, tag="bb2r")
            ones_row = pp.tile([1, P], FP32, tag="ones_row")

            kdram = dpool.tile([HP, P, B, S], FP8)               # K^T spill

            eps_sb = pp.tile([P, 1], FP32, tag="eps")
            nc.vector.memset(eps_sb[:], LN_EPS)
            nc.vector.memset(ones_row[:], 1.0)
            nc.gpsimd.dma_start(bqs_sb[:], bqs_t)
            nc.gpsimd.dma_start(bb1_sb[:], bb1_t)
            nc.gpsimd.dma_start(bb2r_sb[:], bb2r[:])

            with tc.tile_pool(name="x1p", bufs=1) as x1p:
                # alive A..F
                x1_sb = x1p.tile([P, B, B, D], FP32, tag="x1")
                x1T_sb = x1p.tile([P, KT, B, CH], BF16, tag="x1T")

                with tc.tile_pool(name="bigp", bufs=1) as bigp:
                  # one big slot reused across phases: V (A..B), then w2 (C..F)
                  v_sb = bigp.tile([P, B, TT, H, VW], BF16, tag="big")
                  with tc.tile_pool(name="otx", bufs=1) as otx:
                    # alive A..C (1 MB)
                    oT_sb = otx.tile([P, MT, B, CH], FP8, tag="oT")

                    with tc.tile_pool(name="vq", bufs=1) as vq:
                        # zero-padded Q^T: per head-pair, head0 queries in
                        # cols 0:CH (partitions 64:128 zero), head1 queries
                        # in cols CH:2CH (partitions 0:64 zero). One scores
                        # matmul then serves BOTH heads with free=512: the
                        # complementary kpair rows hit exact zeros.
                        qT_sb = vq.tile([P, HP, B, 2 * CH], BF16, tag="qT")
                        k0_sb = vq.tile([P, B, S], FP8, tag="k0")
                        bvr_sb = vq.tile([P, D], FP32, tag="bvr")
                        ones_col = vq.tile([1, DH], F32R, tag="ones_col")
                        ones_f32 = vq.tile([1, DH], FP32, tag="ones_f32")
                        nc.gpsimd.dma_start(bvr_sb[:], bv_rep[:])
                        nc.vector.memset(ones_f32[:], 1.0)
                        with nc.allow_low_precision(
                                reason="f32r ones for 1-cyc/row broadcast"):
                            nc.vector.tensor_copy(ones_col[:], ones_f32[:])
                        # ones lane for the fused softmax row-sum
                        # (on the otherwise-idle gpsimd engine)
                        nc.gpsimd.memset(
                            v_sb[:, :, :, :, ds(DH, 1)], 1.0)

                        # ===== Phase A: projections (K^T, V, Q^T) =====
                        # aout outlives A: its ksb tiles' last readers are
                        # kdram-write DMAs that drain late; keeping the
                        # pool open stops B's pools from WAR-waiting on
                        # that space.
                        apool_cm = tc.tile_pool(name="aout", bufs=6)
                        apool = apool_cm.__enter__()
                        with (
                            tc.tile_pool(name="wqp", bufs=1) as wqpool,
                            tc.tile_pool(name="wqkv", bufs=2) as wpool,
                            tc.tile_pool(name="xt", bufs=3) as xpool,
                            tc.tile_pool(name="psA", bufs=5,
                                         space="PSUM") as psA,
                        ):
                            # initial loads fan out over the DMA queues
                            wq_sb = wqpool.tile([P, KT, D], FP8, tag="wq")
                            nc.gpsimd.dma_start(wq_sb[:], wq_t)
                            wk_sb = wpool.tile([P, KT, D], FP8, tag="w")
                            nc.sync.dma_start(wk_sb[:], wk_t)
                            wv_sb = wpool.tile([P, KT, D], FP8, tag="w")
                            nc.scalar.dma_start(wv_sb[:], wv_t)
                            xtq_sb = xpool.tile([P, KT, B, CH], FP8,
                                                tag="xtq")
                            nc.gpsimd.dma_start(xtq_sb[:], xTq_t)

                            # Q^T first (only needs wq+xtq): its DVE
                            # evacs land early so phase B's first scores
                            # don't wait on A's whole DVE queue, and the
                            # matmuls fill the initial weight-DMA window.
                            nc.gpsimd.memset(
                                qT_sb[ds(0, DH), :, :, ds(CH, CH)], 0.0)
                            nc.gpsimd.memset(
                                qT_sb[ds(DH, DH), :, :, ds(0, CH)], 0.0)
                            for m in range(MT):
                                # both batches per matmul: free=512 halves
                                # the ldweights count
                                ps = psA.tile([P, B, CH], FP32, tag="psq",
                                              bufs=2)
                                for kt in range(0, KT, 2):
                                    nc.tensor.matmul(
                                        ps[:],
                                        wq_sb[:, ds(kt, 2), ts(m, P)],
                                        xtq_sb[:, ds(kt, 2), :, :],
                                        start=(kt == 0),
                                        stop=(kt == KT - 2),
                                        perf_mode=DR)
                                for b in range(B):
                                    for h01 in range(2):
                                        po = h01 * DH
                                        nc.vector.tensor_scalar(
                                            qT_sb[ds(po, DH), m, b,
                                                  ds(h01 * CH, CH)],
                                            ps[ds(po, DH), b, :],
                                            0.125 * WSI,
                                            bqs_sb[ds(po, DH), ds(m, 1)],
                                            ALU.mult, ALU.add)

                            SQ = S // 4  # 512-token stream chunks
                            for b in range(B):
                                for sh in range(4):
                                    xth = xpool.tile([P, KT, SQ], FP8,
                                                     tag="xth")
                                    (nc.scalar if (b + sh) == 0 else
                                     nc.sync).dma_start(
                                        xth[:],
                                        xT_t[:, :, b, ds(sh * SQ, SQ)])
                                    # K^T and V interleaved so the Act
                                    # (K-evac) and DVE (V-evac) engines
                                    # alternate and psum banks recycle
                                    # without bursty evac lag.
                                    # NOTE: bk is dropped on purpose:
                                    # K's bias adds q.bk to every score
                                    # of a query, which softmax cancels.
                                    for m in range(MT):
                                        ps = psA.tile([P, 512], FP32,
                                                      tag="psa")
                                        for kt in range(0, KT, 2):
                                            nc.tensor.matmul(
                                                ps[:],
                                                wk_sb[:, ds(kt, 2),
                                                      ts(m, P)],
                                                xth[:, ds(kt, 2), :],
                                                start=(kt == 0),
                                                stop=(kt == KT - 2),
                                                perf_mode=DR)
                                        if m == 0:
                                            nc.scalar.activation(
                                                k0_sb[:, b, ds(sh * SQ, SQ)],
                                                ps[:], AF.Copy,
                                                bias=0.0, scale=WSI)
                                        else:
                                            ksb = apool.tile([P, 512], FP8,
                                                             tag="ksb")
                                            nc.scalar.activation(
                                                ksb[:], ps[:], AF.Copy,
                                                bias=0.0, scale=WSI)
                                            nc.gpsimd.dma_start(
                                                kdram[m, :, b,
                                                      ds(sh * SQ, SQ)],
                                                ksb[:])
                                        # V (token-major, 65-lane layout)
                                        tl, nb = divmod(m, D // 512)
                                        tt = sh * (SQ // P) + tl
                                        ps = psA.tile([P, 512], FP32,
                                                      tag="psa")
                                        for kt in range(0, KT, 2):
                                            nc.tensor.matmul(
                                                ps[:],
                                                xth[:, ds(kt, 2),
                                                    ts(tl, P)],
                                                wv_sb[:, ds(kt, 2),
                                                      ts(nb, 512)],
                                                start=(kt == 0),
                                                stop=(kt == KT - 2),
                                                perf_mode=DR)
                                        nc.vector.scalar_tensor_tensor(
                                            v_sb[:, b, tt,
                                                 ds(nb * 8, 8),
                                                 ds(0, DH)],
                                            ps[:], WSI,
                                            bvr_sb[:, ts(nb, 512)],
                                            ALU.mult, ALU.add)

                        # ===== Phase B: attention =====
                        # prefetch phase-C weights while attention runs
                        with tc.tile_pool(name="wo_p", bufs=1) as wopool:
                            wo_sb = wopool.tile([P, KT, D], FP8, tag="wo")
                            nc.scalar.dma_start(wo_sb[:], wo_t)
                            ident = wopool.tile([P, P], FP32, tag="ident")
                            nc.scalar.dma_start(ident[:], identd[:])
                            g1r_sb = wopool.tile([P, D], FP32, tag="g1r")
                            b1r_sb = wopool.tile([P, D], FP32, tag="b1r")
                            nc.gpsimd.dma_start(g1r_sb[:], g1_rep[:])
                            nc.gpsimd.dma_start(b1r_sb[:], b1_rep[:])

                            if "B" not in phases:
                                nc.vector.memset(oT_sb[:], 0.001)
                            with (
                                tc.tile_pool(name="kpair", bufs=3) as kpool,
                                tc.tile_pool(name="expst", bufs=2) as epool,
                                tc.tile_pool(name="battn", bufs=4) as bpool,
                                tc.tile_pool(name="psS", bufs=2,
                                             space="PSUM") as psS,
                                tc.tile_pool(name="psO", bufs=3,
                                             space="PSUM") as psO,
                                tc.tile_pool(name="psR", bufs=1,
                                             space="PSUM") as psR,
                            ):
                                hpb = [(b, hp) for b in range(B)
                                       for hp in range(HP)]
                                for b, hp in (hpb if "B" in phases else ()):
                                    if True:
                                        if hp == 0:
                                            kpair = k0_sb[:, b, :]
                                        else:
                                            kp_t = kpool.tile([P, S], FP8,
                                                              tag="kpair")
                                            nc.sync.dma_start(
                                                kp_t[:], kdram[hp, :, b, :])
                                            kpair = kp_t
                                        # scores for BOTH heads at once:
                                        # kpair full 128 partitions vs the
                                        # zero-padded qT -> S^T_h0 in cols
                                        # 0:CH, S^T_h1 in cols CH:2CH.
                                        expst = epool.tile(
                                            [P, TT, 2 * CH], BF16,
                                            tag="expst")
                                        EG2 = 2
                                        for g in range(TT // EG2):
                                            pss = psS.tile(
                                                [P, EG2, 2 * CH], FP32,
                                                tag="pss")
                                            for j in range(EG2):
                                                tt = g * EG2 + j
                                                nc.tensor.matmul(
                                                    pss[:, j, :],
                                                    kpair[:, ts(tt, P)],
                                                    qT_sb[:, hp, b, :])
                                            nc.scalar.activation(
                                                expst[:, ds(g * EG2, EG2),
                                                      :],
                                                pss[:], AF.Exp)
                                        # PV: interleave the two heads'
                                        # psum accumulations so same-bank
                                        # back-to-back writes don't pay
                                        # the PE->PSUM drain each step
                                        po_a = psO.tile([VW, CH], FP32,
                                                        tag="pso")
                                        po_b = psO.tile([VW, CH], FP32,
                                                        tag="pso")
                                        po_both = [po_a, po_b]
                                        for tt in range(TT):
                                            for h01 in range(2):
                                                h = hp * 2 + h01
                                                nc.tensor.matmul(
                                                    po_both[h01][:],
                                                    v_sb[:, b, tt, h, :],
                                                    expst[:, tt,
                                                          ds(h01 * CH, CH)],
                                                    start=(tt == 0),
                                                    stop=(tt == TT - 1))
                                        for h01 in range(2):
                                            po = h01 * DH
                                            h = hp * 2 + h01
                                            po_ps = po_both[h01]
                                            # softmax denominators sit in
                                            # row 64 (the ones lane)
                                            rsum = bpool.tile([1, CH], FP32,
                                                              tag="rsum")
                                            nc.vector.tensor_copy(
                                                rsum[:],
                                                po_ps[ds(DH, 1), :])
                                            rec = bpool.tile([1, CH], F32R,
                                                             tag="rec")
                                            with nc.allow_low_precision(
                                                    reason="f32r recip for "
                                                    "1-cyc/row broadcast"):
                                                nc.vector.reciprocal(
                                                    rec[:], rsum[:])
                                            # broadcast across partitions:
                                            # ones[1,64]^T @ rec[1,CH]
                                            recb = psR.tile([DH, CH], FP32,
                                                            tag="recb")
                                            nc.tensor.matmul(
                                                recb[:], ones_col[:],
                                                rec[:],
                                                start=True, stop=True)
                                            recb_sb = bpool.tile(
                                                [DH, CH], FP32, tag="recb_sb")
                                            nc.vector.tensor_copy(
                                                recb_sb[:], recb[:])
                                            with nc.allow_low_precision(
                                                    reason="oT stored fp8; "
                                                    "noise hidden by "
                                                    "residual LN"):
                                                nc.vector.tensor_tensor(
                                                    oT_sb[ds(po, DH),
                                                          hp, b, :],
                                                    po_ps[ds(0, DH), :],
                                                    recb_sb[:], ALU.mult)

                    # ===== Phase C: O-proj + residual + LN1 (+ x1^T) =====
                            # w2 takes over V's SBUF slot; chunks are
                            # paced through C and E to avoid a DMA spike
                            w2r_sb = bigp.tile([P, FT // 2, 2, D + 16],
                                               BF16, tag="big")
                            if "C" not in phases:
                                nc.vector.memset(x1_sb[:], 0.001)
                                nc.vector.memset(x1T_sb[:], 0.001)
                            with (
                                tc.tile_pool(name="cscr", bufs=2) as cpool,
                                tc.tile_pool(name="psC", bufs=2,
                                             space="PSUM") as psC,
                                tc.tile_pool(name="psD", bufs=2,
                                             space="PSUM") as psD,
                            ):
                                for b in range(B if "C" in phases else 0):
                                    for tq in range(B):
                                        kp = b * B + tq
                                        nc.gpsimd.dma_start(
                                            w2r_sb[:, kp, :, ds(0, D)],
                                            w2s[kp])
                                        ps = psC.tile([P, D], FP32, tag="psc")
                                        for nb in range(D // 512):
                                            for kt in range(0, KT, 2):
                                                nc.tensor.matmul(
                                                    ps[:, ts(nb, 512)],
                                                    oT_sb[:, ds(kt, 2), b,
                                                          ts(tq, P)],
                                                    wo_sb[:, ds(kt, 2),
                                                          ts(nb, 512)],
                                                    start=(kt == 0),
                                                    stop=(kt == KT - 2),
                                                    perf_mode=DR)
                                        xres_sb = cpool.tile([P, D], FP32,
                                                             tag="xres")
                                        nc.sync.dma_start(
                                            xres_sb[:], xres_t[:, b, tq, :])
                                        # fused: t1 = ps + (x + bo),
                                        # row-sum for the LN mean
                                        t1 = cpool.tile([P, D], FP32,
                                                        tag="c_t1")
                                        ssum = cpool.tile([P, 1], FP32,
                                                          tag="c_ssum")
                                        nc.vector.scalar_tensor_tensor(
                                            t1[:], ps[:], WSI, xres_sb[:],
                                            ALU.mult, ALU.add,
                                            accum_out=ssum[:])
                                        y_t = layer_norm_tail(
                                            cpool, t1, ssum, g1r_sb, b1r_sb,
                                            x1_sb[:, b, tq, :])
                                        # transposes take the PRE-affine y:
                                        # g1 is folded into W1 host-side
                                        # (bb1' = bb1 + b1@W1), so FFN1 is
                                        # exact while the affine x1 (for
                                        # the F residual) finishes in
                                        # parallel.
                                        for kd in range(KT):
                                            pt = psD.tile([P, P], FP32,
                                                          tag="psd")
                                            nc.tensor.transpose(
                                                pt[:],
                                                y_t[:, ts(kd, P)],
                                                ident[:])
                                            nc.scalar.copy(
                                                x1T_sb[:, kd, b, ts(tq, P)],
                                                pt[:])
                        apool_cm.__exit__(None, None, None)

                  # ===== Phase E: FFN1  hT = relu(x1@W1+bb1)^T =====
                  with tc.tile_pool(name="hT", bufs=1) as hpool:
                      hT_sb = hpool.tile([P, FT, TQ], BF16, tag="hT")
                      with (
                          tc.tile_pool(name="w1_p", bufs=4) as w1pool,
                          tc.tile_pool(name="psE", bufs=4,
                                       space="PSUM") as psE,
                      ):
                          if "E" not in phases:
                              nc.vector.memset(hT_sb[:], 0.001)
                          for mh in range(FT if "E" in phases else 0):
                              if mh % 2 == 0 and 4 + mh // 2 < FT // 2:
                                  kp = 4 + mh // 2
                                  nc.gpsimd.dma_start(
                                      w2r_sb[:, kp, :, ds(0, D)],
                                      w2s[kp])
                              w1t = w1pool.tile([P, KT, P], BF16, tag="w1t")
                              nc.sync.dma_start(w1t[:], w1s[mh])
                              ps = psE.tile([P, TQ], FP32, tag="pse")
                              for kt in range(KT):
                                  nc.tensor.matmul(
                                      ps[:], w1t[:, kt, :],
                                      x1T_sb[:, kt, :, :],
                                      start=(kt == 0), stop=(kt == KT - 1))
                              nc.scalar.activation(
                                  hT_sb[:, mh, :], ps[:], AF.Relu,
                                  bias=bb1_sb[:, ds(mh, 1)])

                      # ===== Phase F: FFN2 + residual + LN2 -> out =====
                      # kt-outer, two token-halves; w2 streamed per kt-pair
                      with (
                          tc.tile_pool(name="fscr", bufs=2) as fpool,
                          tc.tile_pool(name="fbias", bufs=1) as fbp,
                          tc.tile_pool(name="psF", bufs=2, space="PSUM") as psF,
                      ):
                          g2r_sb = fbp.tile([P, D], FP32, tag="g2r")
                          b2r_sb = fbp.tile([P, D], FP32, tag="b2r")
                          nc.gpsimd.dma_start(g2r_sb[:], g2_rep[:])
                          nc.gpsimd.dma_start(b2r_sb[:], b2_rep[:])
                          if "F" not in phases:
                              for b in range(B):
                                  for tq in range(B):
                                      dummy = fpool.tile([P, D], FP32,
                                                         tag="f_out")
                                      nc.vector.memset(dummy[:], 0.5)
                                      nc.sync.dma_start(out_t[:, b, tq, :],
                                                        dummy[:])
                          for c in range(B * B if "F" in phases else 0):
                              ps_f = psF.tile([P, D], FP32, tag=f"psf{c % 2}")
                              for kp in range(FT // 2):
                                  for kj in range(2):
                                      for nb in range(D // 512):
                                          nc.tensor.matmul(
                                              ps_f[:, ts(nb, 512)],
                                              hT_sb[:, 2 * kp + kj,
                                                    ts(c, P)],
                                              w2r_sb[:, kp, kj, ts(nb, 512)],
                                              start=(kp == 0 and kj == 0),
                                              stop=False)
                              # bb2 via contract-1 matmuls on the idle
                              # PE; closes the accumulation group
                              for nb in range(D // 512):
                                  nc.tensor.matmul(
                                      ps_f[:, ts(nb, 512)],
                                      ones_row[:],
                                      bb2r_sb[:, ts(nb, 512)],
                                      start=False,
                                      stop=True)
                              if True:
                                  b, tq = divmod(c, B)
                                  t1 = fpool.tile([P, D], FP32, tag="f_t1")
                                  ssum = fpool.tile([P, 1], FP32,
                                                    tag="f_ssum")
                                  nc.vector.scalar_tensor_tensor(
                                      t1[:], ps_f[:], 1.0,
                                      x1_sb[:, b, tq, :],
                                      ALU.mult, ALU.add, accum_out=ssum[:])
                                  o_sb = fpool.tile([P, D], FP32, tag="f_out")
                                  layer_norm_tail(fpool, t1, ssum,
                                                  g2r_sb, b2r_sb, o_sb,
                                                  out_halves=(out_t, b, tq))

    if split_waits:
        _split_multiwaits(nc)
    return nc


_NC_CACHE = None


def _get_bass():
    global _NC_CACHE
    if _NC_CACHE is None:
        _NC_CACHE = build_bass()
    return _NC_CACHE


def make_in_maps(x, Wq, bq, Wk, bk, Wv, bv, Wo, bo, g1, b1, W1, bb1, W2, bb2,
                 g2, b2):
    bf = ml_dtypes.bfloat16
    f8 = ml_dtypes.float8_e4m3
    x = np.asarray(x, np.float32)
    xT = np.ascontiguousarray(x.transpose(2, 1, 0))              # [D,B,S]
    W1 = np.asarray(W1, np.float32)
    W2 = np.asarray(W2, np.float32)
    g1f = np.asarray(g1, np.float32)
    b1f = np.asarray(b1, np.float32)
    bb1 = np.asarray(bb1, np.float32) + b1f @ W1
    W1 = g1f[:, None] * W1
    # w1s[mh] = W1[:, mh*128:(mh+1)*128] rearranged [(kt p), n] -> [p kt n]
    w1s = np.ascontiguousarray(
        W1.reshape(KT, P, FT, P).transpose(2, 1, 0, 3)).astype(bf)
    # w2s[kp] = W2[kp*256:(kp+1)*256, :] as [P, 2, D]
    w2s = np.ascontiguousarray(
        W2.reshape(FT // 2, 2, P, D).transpose(0, 2, 1, 3)).astype(bf)
    shared = {
        "xT8": xT.astype(f8),
        "wq8": (np.asarray(Wq, np.float32) * WS).astype(f8),
        "wk8": (np.asarray(Wk, np.float32) * WS).astype(f8),
        "wv8": (np.asarray(Wv, np.float32) * WS).astype(f8),
        "wo8": (np.asarray(Wo, np.float32) * WS).astype(f8),
        "w1s": w1s,
        "w2s": w2s,
        "ident": np.eye(P, dtype=np.float32),
        "bqs": (np.asarray(bq, np.float32) / 8.0),
        "bb1": bb1,
        "bb2r": np.asarray(bb2, np.float32).reshape(1, D),
        "bv_rep": np.tile(np.asarray(bv, np.float32), (P, 1)),
        "g1_rep": np.tile(np.asarray(g1, np.float32), (P, 1)),
        "b1_rep": np.tile(np.asarray(b1, np.float32), (P, 1)),
        "g2_rep": np.tile(np.asarray(g2, np.float32), (P, 1)),
        "b2_rep": np.tile(np.asarray(b2, np.float32), (P, 1)),
    }
    xf8 = xT.astype(f8)
    xpbo = x + np.asarray(bo, np.float32)       # fold bo into the residual
    in_maps = []
    for c in range(NC):
        sl = slice(c * CH, (c + 1) * CH)
        m = dict(shared)
        m["xTq8"] = np.ascontiguousarray(xf8[:, :, sl])
        m["xres"] = np.ascontiguousarray(
            xpbo[sl].transpose(1, 0, 2))           # [B, CH, D]
        in_maps.append(m)
    return in_maps


def assemble(results):
    out = np.empty((S, B, D), np.float32)
    for c, r in enumerate(results):
        out[c * CH:(c + 1) * CH] = r["out"].transpose(1, 0, 2)
    return out


def kernel(**inputs) -> np.ndarray:
    nc = _get_bass()
    in_maps = make_in_maps(**inputs)
    res = run_bass_kernel_spmd(nc, in_maps, core_ids=list(range(NC)))
    return assemble(res.results)

